# revision 30
# baseline (speedup 1.0000x reference)
"""Trainium2 Bass kernel for nn_ChannelMambaBlock (v2).

Math (per pixel, channel vector x of size C=192):
  xn  = LN(x)*nw + nb
  p   = w_in @ xn              ; x1, x2 = p[:C], p[C:]
  u   = ssm_in @ x1            ; xss, z = silu(u[:C]), silu(u[C:])
  xd  = x_proj @ xss (K dirs)  ; dtr, Bv, Cv
  dt  = softplus(dt_w @ dtr + dt_b)
  bc_k = sum_s Bv*Cv ; gain = sum_k bc_k*dt_k + sum_k D_k
  y   = LN2(xss*gain)*ow + ob ; y *= z
  s   = ssm_out @ y ; o = w_out @ (s * silu(x2)) ; out = x + o

v2 kernel strategy (channel-major [C, pixels], 8-way data parallel,
16 tiles of 512 pixels per core):
  - ONE ACT table (silu_and_others: Silu/Tanh/Square/Copy). Every
    activation is fused: silu(psum+bias) evacuates the big-matmul psum
    chunks directly; softplus(x) = silu(x) + a*(1-tanh(c*x)^2) with
    fitted a,c (rmse 2e-4); rsqrt via Newton iterations on DVE
    (linear first step folds the constant seed), no Exp/Ln anywhere.
  - LN stats: PE ones-matmuls; Sum(x) from f32r x (no bf16 copy),
    Sum(x^2) from ACT Square tiles. Per-tile [2,512] psum stats are
    evacuated and DMA-stacked into [4,1024] group tiles so the scalar
    chain (mu/var/rsqrt/mu*rstd) runs once per 4 tiles.
  - xd in ONE 112-row matmul [dtr48;Bv32;Cv32]; Cv realigned onto Bv's
    partitions by an SBUF->SBUF DMA (DMA moves across partitions; DVE
    cannot).
  - dt block-diag matmul in c-major row order (row c*4+k); gain k-sum
    via 0/1 selector matmuls; bc broadcast to the c*4+k pattern with
    one selector matmul (as baseline).
  - out_norm shift handled with sign trick: w = (mb2-obp)-y*rstd2,
    v = w*z, s = (-Wsp')@v.
"""

import numpy as np

C = 192
K = 4
DT = 12
DS = 8
B, H, W = 4, 128, 128
EPS = 1e-5
NCORES = 8
NPIX = B * H * W // NCORES      # 8192 per core
NT = 512                        # pixels per tile
NTILES = NPIX // NT             # 16
GRP = 4                         # tiles per LN-scalar batch
NGRP = NTILES // GRP

# M-chunks of the big fused matmul [u(384); x2(192)]:
QCH = [(0, 128), (128, 192), (192, 320), (320, 384), (384, 512), (512, 576)]

# softplus(x) ~= silu(x) + SP_A*(1 - tanh(SP_C*x)^2)
SP_A = 0.6930162
SP_C = 0.4230186
# rsqrt Newton: y1 = 1.5*s - 0.5*s^3*v (linear in v), then iterate
SEED1, IT1 = 0.82, 3
SEED2, IT2 = 0.10, 11

NPCOL = 22

_CACHE = {}


def _fold_weights(norm_w, norm_b, w_in, ssm_in_w, x_proj_w, dt_w, dt_b,
                  A_logs, Ds, out_norm_w, out_norm_b, ssm_out_w, w_out):
    f8 = np.float64
    nw, nb = norm_w.astype(f8), norm_b.astype(f8)
    w_in = w_in.astype(f8)
    ssm = ssm_in_w.astype(f8)
    W1 = w_in * nw[None, :]
    b1 = w_in @ nb
    W_u = ssm @ W1[:C]                      # (2C, C)
    b_u = ssm @ b1[:C]
    W_big = np.concatenate([W_u, W1[C:]], 0)   # (576, C)
    b_big = np.concatenate([b_u, b1[C:]], 0)   # (576,)
    # xd rows: [dtr (48, k-major) ; pad (16) ; Bv (32) ; Cv (32)] = 128
    Xp = np.concatenate([
        x_proj_w[:, :DT].reshape(K * DT, C),
        np.zeros((16, C)),
        x_proj_w[:, DT:DT + DS].reshape(K * DS, C),
        x_proj_w[:, DT + DS:].reshape(K * DS, C)], 0).astype(f8)   # (128, C)
    # dt block: out row = c*4 + k, in col = k*12 + r
    Wdt = np.zeros((C * K, K * DT))
    for k in range(K):
        Wdt[np.arange(C) * K + k, k * DT:(k + 1) * DT] = dt_w[k].astype(f8)
    dtb_s = dt_b.astype(f8).T.reshape(C * K)  # row c*4+k
    Dsum = Ds.astype(f8).reshape(K, C).sum(0)
    ow, ob = out_norm_w.astype(f8), out_norm_b.astype(f8)
    Wsp_neg = -(ssm_out_w.astype(f8) * ow[None, :])
    obp = ob / ow
    Wo = w_out.astype(f8)

    # bc pattern: bcm[m] = bc[m%4]; bcpt rows at partitions 64:96
    combT = np.zeros((96, 128))
    for k in range(K):
        rows = 64 + k * DS + np.arange(DS)
        combT[np.ix_(rows, np.arange(128)[np.arange(128) % 4 == k])] = 1.0
    # gain selectors: chunk j of dt rows [128j,128j+128) covers c in
    # [32j, 32j+32): gain_sel_j[p, 32j + p//4] = 1
    selsA = []
    for j in range(4):
        S = np.zeros((128, 128))
        S[np.arange(128), 32 * j + np.arange(128) // 4] = 1.0
        selsA.append(S)
    selsB = []
    for j in range(2):
        S = np.zeros((128, 64))
        S[np.arange(128), 32 * j + np.arange(128) // 4] = 1.0
        selsB.append(S)

    def f32(a):
        return np.ascontiguousarray(np.asarray(a, np.float32))

    import ml_dtypes

    def bf(a):
        return np.ascontiguousarray(np.asarray(a).astype(ml_dtypes.bfloat16))

    wts = {
        "wbigT": bf(W_big.T),               # (192, 576)
        "xpT": bf(Xp.T),                    # (192, 128)
        "wdtT": bf(Wdt.T),                  # (48, 768)
        "combT": bf(combT),                 # (96, 128)
        "selsA": bf(np.concatenate(selsA, 1)),   # (128, 512)
        "selsB": bf(np.concatenate(selsB, 1)),   # (128, 128)
        "wspT": bf(Wsp_neg.T),              # (192, 192)  (negated!)
        "woT": bf(Wo.T),                    # (192, 192)
        "onescb": bf(np.ones((C, 1))),      # (192, 1) stats lhsT (bf16)
        "ident": f32(np.eye(128)),          # transpose rhs
    }
    cols = []

    def col(v):
        v = np.asarray(v, np.float64).reshape(-1)
        c = np.zeros(128)
        c[:v.size] = v
        cols.append(c)
        return len(cols) - 1

    ci = {}
    for m, (r0, r1) in enumerate(QCH):
        ci[f"b{m}"] = col(b_big[r0:r1])
    for j in range(6):
        ci[f"dtb{j}"] = col(dtb_s[128 * j:128 * (j + 1)])
        ci[f"dtbc{j}"] = col(SP_C * dtb_s[128 * j:128 * (j + 1)])
    ci["DsA"] = col(Dsum[:128]); ci["DsB"] = col(Dsum[128:])
    ci["obpA"] = col(obp[:128]); ci["obpB"] = col(obp[128:])
    assert len(cols) == NPCOL, len(cols)
    wts["pcol"] = f32(np.stack(cols, 1))     # (128, NPCOL)
    return wts, ci


def _register_customs():
    """Custom DVE ops:
      VAR_EPS_ANT:  out = in0*s0 + s1 - in1^2           (var from sums)
      NEWTON_RS_ANT: out = in1*(s0 - s1*in0*in1^2)      (rsqrt Newton step)
      SPCOMB_ANT:   out = in0 + s0 - s0*in1^2           (softplus combine)
    """
    import concourse.dve_ops as do
    from concourse.dve_spec import Spec, Src0, Src1, C0, C1, sq, lower
    from concourse.dve_uop import DveOpSpec
    if hasattr(do, "VAR_EPS_ANT"):
        return do.VAR_EPS_ANT, do.NEWTON_RS_ANT, do.SPCOMB_ANT

    def mk(name, body, ref):
        spec = Spec(body=body, reference=ref)
        opcode = do._CUSTOM_DVE_ROW_BASE + len(do.OPS)
        shas = {}
        for ver in ("v3", "v4"):
            s = DveOpSpec(name=name, opcode=opcode,
                          uops=lower(spec, ver=ver), rd1_en=True)
            shas[ver] = s.sha(ver)
        op = do.DveOp(name, spec, subdim=False, uops_sha=shas)
        do.OPS.append(op)
        do._SUB_OPCODE_FOR_NAME[op.name] = opcode
        do.CUSTOM_DVE_SPECS[op.name] = spec
        setattr(do, name, op)
        return op

    var_op = mk("VAR_EPS_ANT",
                Src0 * C0 + C1 - sq(Src1),
                lambda in0, in1, s0, s1, imm2:
                (in0.astype(np.float32) * s0 + s1 - in1 * in1))
    newt = mk("NEWTON_RS_ANT",
              Src1 * (C0 - C1 * Src0 * sq(Src1)),
              lambda in0, in1, s0, s1, imm2:
              (in1.astype(np.float32)
               * (s0 - s1 * in0 * in1 * in1)))
    spc = mk("SPCOMB_ANT",
             Src0 + C0 - C0 * sq(Src1),
             lambda in0, in1, s0, s1, imm2:
             (in0.astype(np.float32) + s0 - s0 * in1 * in1))
    return var_op, newt, spc


def _patch_act_tables():
    """Pin every activation we use (Silu/Tanh/Square/Copy/Identity) to the
    single silu_and_others table so one hoisted ACT table load serves the
    whole kernel."""
    import concourse.mybir as mybir
    import concourse.bacc as bacc_mod
    import concourse.hw_specs as hw
    if getattr(bacc_mod, "_ant_act_tables_patched", False):
        return
    orig = hw.get_activation_tables
    AF = mybir.ActivationFunctionType
    keep = "silu_and_others"
    mine = {AF.Silu, AF.Tanh, AF.Square, AF.Copy, AF.Identity}

    def patched(arch):
        t = dict(orig(arch))
        for name in list(t.keys()):
            if name != keep:
                t[name] = t[name] - mine
        return t

    bacc_mod.get_activation_tables = patched
    bacc_mod._ant_act_tables_patched = True


def _build_program():
    import concourse.mybir as mybir
    from concourse.bacc import Bacc
    from concourse.tile import TileContext
    from concourse.alu_op_type import AluOpType

    _patch_act_tables()
    var_op, newt_op, spc_op = _register_customs()

    AF = mybir.ActivationFunctionType
    OP = AluOpType
    f32 = mybir.dt.float32
    f32r = mybir.dt.float32r
    bf = mybir.dt.bfloat16
    nc = Bacc()

    xin = nc.dram_tensor("xin", (C, NPIX), f32r, kind="ExternalInput")
    yout = nc.dram_tensor("yout", (C, NPIX), f32, kind="ExternalOutput")
    dw = {}
    for name, shape in [("wbigT", (C, 576)), ("xpT", (C, 128)),
                        ("wdtT", (48, 768)), ("combT", (96, 128)),
                        ("selsA", (128, 512)), ("selsB", (128, 128)),
                        ("wspT", (C, 192)), ("woT", (C, 192))]:
        dw[name] = nc.dram_tensor(name, shape, bf, kind="ExternalInput")
    dw["onescb"] = nc.dram_tensor("onescb", (C, 1), bf, kind="ExternalInput")
    dw["ident"] = nc.dram_tensor("ident", (128, 128), f32, kind="ExternalInput")
    xpm = nc.dram_tensor("xpm", (NPIX // 128, 128, C), bf,
                         kind="ExternalInput")
    dw["pcol"] = nc.dram_tensor("pcol", (128, NPCOL), f32, kind="ExternalInput")
    nrm1d = nc.dram_tensor("nrm1d", (NTILES, 2 * NT), bf, kind="Internal")
    nrm2d = nc.dram_tensor("nrm2d", (NTILES, 2 * NT), bf, kind="Internal")

    with TileContext(nc) as tc:
        with (
            tc.tile_pool(name="wpool", bufs=1) as wp,
            tc.tile_pool(name="work", bufs=1) as wk,
            tc.tile_pool(name="ps_q", bufs=3, space="PSUM") as ps_q,     # q + dt
            tc.tile_pool(name="ps_mm2", bufs=2, space="PSUM") as ps_mm2,  # gain/s/o
            tc.tile_pool(name="ps_xd", bufs=2, space="PSUM") as ps_xd,   # xd/bcm
            tc.tile_pool(name="ps_st", bufs=1, space="PSUM") as ps_st,   # stats
        ):
            # ---- load weights once ----
            wt = {}
            for name, shape in [("wbigT_lo", (128, 576)), ("wbigT_hi", (64, 576)),
                                ("xpT_lo", (128, 128)), ("xpT_hi", (64, 128)),
                                ("wdtT", (48, 768)), ("combT", (96, 128)),
                                ("selsA", (128, 512)), ("selsB", (128, 128)),
                                ("wspT_lo", (128, 192)), ("wspT_hi", (64, 192)),
                                ("woT_lo", (128, 192)), ("woT_hi", (64, 192)),
                                ("onesb_lo", (128, 1)), ("onesb_hi", (64, 1))]:
                wt[name] = wp.tile(list(shape), bf, tag=name, name=name)
            wt["ident"] = wp.tile([128, 128], f32, tag="ident", name="ident")
            wt["pcol"] = wp.tile([128, NPCOL], f32, tag="pcol", name="pcol")
            for nm, src in [("wbigT_lo", dw["wbigT"][0:128, :]),
                            ("wbigT_hi", dw["wbigT"][128:192, :]),
                            ("xpT_lo", dw["xpT"][0:128, :]),
                            ("xpT_hi", dw["xpT"][128:192, :]),
                            ("wdtT", dw["wdtT"][:]),
                            ("combT", dw["combT"][:]),
                            ("selsA", dw["selsA"][:]),
                            ("selsB", dw["selsB"][:]),
                            ("wspT_lo", dw["wspT"][0:128, :]),
                            ("wspT_hi", dw["wspT"][128:192, :]),
                            ("woT_lo", dw["woT"][0:128, :]),
                            ("woT_hi", dw["woT"][128:192, :]),
                            ("ident", dw["ident"][:]),
                            ("onesb_lo", dw["onescb"][0:128, :]),
                            ("onesb_hi", dw["onescb"][128:192, :]),
                            ("pcol", dw["pcol"][:])]:
                nc.sync.dma_start(wt[nm][:], src)

            PC = {}
            idx = 0
            for m in range(6):
                PC[f"b{m}"] = idx; idx += 1
            for j in range(6):
                PC[f"dtb{j}"] = idx; idx += 1
                PC[f"dtbc{j}"] = idx; idx += 1
            for nm in ["DsA", "DsB", "obpA", "obpB"]:
                PC[nm] = idx; idx += 1

            def pc(name, rows=128):
                return wt["pcol"][0:rows, PC[name]:PC[name] + 1]

            def batched_rsqrt(batch, rows, seed, iters, tag, uid,
                              r0=0, r1=None, nrm=None):
                """batch [rows,1024] f32 (cols 0:512 sum(x), 512:1024
                sum(x^2)); processes row slice [r0:r1]. Returns nrm
                [rows,1024] bf (cols 0:512 rstd, 512:1024 mu*rstd)."""
                if r1 is None:
                    r1 = rows
                rs = slice(r0, r1)
                mu = wk.tile([rows, NT], f32, tag=f"mu{tag}",
                             name=f"mu{uid}", bufs=1)
                nc.vector.tensor_scalar_mul(mu[rs, :], batch[rs, 0:NT],
                                            1.0 / C)
                var = wk.tile([rows, NT], f32, tag=f"var{tag}",
                              name=f"var{uid}", bufs=1)
                nc.vector._custom_dve(var_op, out=var[rs, :],
                                      in0=batch[rs, NT:2 * NT], in1=mu[rs, :],
                                      s0=1.0 / C, s1=EPS)
                y0 = wk.tile([rows, NT], f32, tag=f"nt{tag}",
                             name=f"y0{uid}", bufs=2)
                nc.vector.tensor_scalar(y0[rs, :], var[rs, :],
                                        -0.5 * seed ** 3,
                                        1.5 * seed, OP.mult, OP.add)
                cur = y0
                if nrm is None:
                    nrm = wk.tile([rows, 2 * NT], bf, tag=f"nrm{tag}",
                                  name=f"nrm{uid}", bufs=2)
                for i in range(iters):
                    last = (i == iters - 1)
                    if last:
                        nc.vector._custom_dve(newt_op, out=nrm[rs, 0:NT],
                                              in0=var[rs, :], in1=cur[rs, :],
                                              s0=1.5, s1=0.5)
                    else:
                        yn = wk.tile([rows, NT], f32, tag=f"nt{tag}",
                                     name=f"y{i + 1}{uid}", bufs=2)
                        nc.vector._custom_dve(newt_op, out=yn[rs, :],
                                              in0=var[rs, :], in1=cur[rs, :],
                                              s0=1.5, s1=0.5)
                        cur = yn
                nc.vector.tensor_tensor(nrm[rs, NT:2 * NT], mu[rs, :],
                                        nrm[rs, 0:NT], OP.mult)
                return nrm

            sx2as, sx2bs, zas, zbs, yas, ybs = {}, {}, {}, {}, {}, {}

            # (LN1 is computed per tile inside phaseC via pixel-major
            # DVE reduces; no prologue needed.)

            # ================= MAIN LOOP =================
            def ln1_tile(t):
                # ---- LN1 stats from pixel-major x (DVE/GPS reduces) ----
                xp = wk.tile([128, 4 * C], bf, tag="xp", name=f"xp{t}", bufs=3)
                nc.sync.dma_start(
                    xp[:], xpm[4 * t:4 * t + 4, :, :].rearrange(
                        "j p c -> p j c"))
                xp3 = xp[:].rearrange("p (j c) -> p j c", j=4)
                sqp = wk.tile([128, 4 * C], bf, tag="sqp", name=f"sqp{t}",
                              bufs=2)
                nc.gpsimd.tensor_mul(sqp[:], xp[:], xp[:])
                ssum = wk.tile([128, 4], f32, tag="ssum", name=f"ssum{t}",
                               bufs=2)
                nc.vector.tensor_reduce(ssum[:].unsqueeze(2),
                                        xp3, mybir.AxisListType.X, OP.add)
                qsum = wk.tile([128, 4], f32, tag="qsum", name=f"qsum{t}",
                               bufs=2)
                nc.vector.tensor_reduce(
                    qsum[:].unsqueeze(2),
                    sqp[:].rearrange("p (j c) -> p j c", j=4),
                    mybir.AxisListType.X, OP.add)
                mu = wk.tile([128, 4], f32, tag="mus", name=f"mus{t}", bufs=2)
                nc.vector.tensor_scalar_mul(mu[:], ssum[:], 1.0 / C)
                var = wk.tile([128, 4], f32, tag="vars", name=f"vars{t}",
                              bufs=2)
                nc.vector._custom_dve(var_op, out=var[:], in0=qsum[:],
                                      in1=mu[:], s0=1.0 / C, s1=EPS)
                cur = wk.tile([128, 4], f32, tag="nts", name=f"y0s{t}", bufs=2)
                nc.vector.tensor_scalar(cur[:], var[:], -0.5 * SEED1 ** 3,
                                        1.5 * SEED1, OP.mult, OP.add)
                pk = wk.tile([128, 8], f32, tag="pks", name=f"pks{t}", bufs=2)
                for it in range(IT1):
                    dst = (pk[:, 0:4] if it == IT1 - 1 else
                           wk.tile([128, 4], f32, tag="nts",
                                   name=f"y{it + 1}s{t}", bufs=2)[:])
                    nc.vector._custom_dve(newt_op, out=dst, in0=var[:],
                                          in1=cur[:], s0=1.5, s1=0.5)
                    cur = dst
                nc.vector.tensor_tensor(pk[:, 4:8], mu[:], pk[:, 0:4],
                                        OP.mult)
                ptr = ps_st.tile([8, 128], f32, tag="st", name=f"ptr{t}")
                nc.tensor.transpose(ptr[:], pk[:], wt["ident"][:])
                ntr = wk.tile([8, 128], bf, tag="ntr", name=f"ntr{t}", bufs=2)
                nc.scalar.copy(ntr[:], ptr[:])
                nc.scalar.dma_start(
                    nrm1d[t:t + 1, 0:NT].rearrange("o (j p) -> o j p", j=4),
                    ntr[0:4, :])
                nc.scalar.dma_start(
                    nrm1d[t:t + 1, NT:2 * NT].rearrange("o (j p) -> o j p",
                                                        j=4),
                    ntr[4:8, :])


            LN1_LEAD = 6
            for t in range(LN1_LEAD):
                ln1_tile(t)

            def phaseC(t, i, batch2):
                ts = slice(t * NT, (t + 1) * NT)
                xc = wk.tile([128, NT], f32r, tag="xl", name=f"xc{t}", bufs=6)
                xch = wk.tile([64, NT], f32r, tag="xh", name=f"xch{t}",
                              bufs=6)
                nc.sync.dma_start(xc[:], xin[0:128, ts])
                nc.sync.dma_start(xch[:], xin[128:192, ts])

                rb = wk.tile([128, NT], bf, tag="rb", name=f"rb{t}", bufs=2)
                mb = wk.tile([128, NT], bf, tag="mb", name=f"mb{t}", bufs=2)
                nc.gpsimd.dma_start(
                    rb[:], nrm1d[t:t + 1, 0:NT].broadcast_to([128, NT]))
                nc.gpsimd.dma_start(
                    mb[:], nrm1d[t:t + 1, NT:2 * NT].broadcast_to([128, NT]))

                # x_hat = x*rstd - mu*rstd
                xt = wk.tile([128, NT], bf, tag="xt", name=f"xt{t}", bufs=2)
                xhl = wk.tile([128, NT], bf, tag="xhl", name=f"xhl{t}", bufs=2)
                xhh = wk.tile([64, NT], bf, tag="xhh", name=f"xhh{t}", bufs=2)
                nc.vector.tensor_tensor(xt[:], xc[:].bitcast(f32), rb[:],
                                        OP.mult)
                nc.vector.tensor_tensor(xhl[:], xt[:], mb[:], OP.subtract)
                nc.vector.tensor_tensor(xt[0:64, :], xch[:].bitcast(f32),
                                        rb[0:64, :], OP.mult)
                nc.vector.tensor_tensor(xhh[:], xt[0:64, :], mb[0:64, :],
                                        OP.subtract)

                # big matmul; silu fused into psum evac
                souts = []
                for m, (r0, r1) in enumerate(QCH):
                    rows = r1 - r0
                    q = ps_q.tile([rows, NT], f32, tag="q", name=f"q{m}_{t}")
                    nc.tensor.matmul(q[:], wt["wbigT_lo"][:, r0:r1], xhl[:],
                                     start=True, stop=False)
                    nc.tensor.matmul(q[:], wt["wbigT_hi"][:, r0:r1], xhh[:],
                                     start=False, stop=True)
                    so = wk.tile([128, NT], bf, tag=f"sil{m}",
                                 name=f"sil{m}_{t}",
                                 bufs=2 if m in (0, 1) else 9)
                    nc.scalar.activation(so[0:rows, :], q[:], AF.Silu,
                                         bias=pc(f"b{m}", rows))
                    souts.append(so)
                xssA, xssB = souts[0], souts[1]
                zas[t], zbs[t] = souts[2], souts[3]
                sx2as[t], sx2bs[t] = souts[4], souts[5]

                # xd = [dtr;pad;Bv;Cv] @ xss  (one 128-row matmul)
                xdp = ps_xd.tile([128, NT], f32, tag="xd", name=f"xd{t}")
                nc.tensor.matmul(xdp[:], wt["xpT_lo"][:], xssA[0:128, :],
                                 start=True, stop=False)
                nc.tensor.matmul(xdp[:], wt["xpT_hi"][:], xssB[0:64, :],
                                 start=False, stop=True)
                xds = wk.tile([128, NT], bf, tag="xds", name=f"xds{t}", bufs=2)
                nc.scalar.copy(xds[:], xdp[:])
                # realign Cv (rows 96:128) onto Bv's partitions 64:96
                cvt = wk.tile([96, NT], bf, tag="cvt", name=f"cvt{t}", bufs=2)
                nc.sync.dma_start(cvt[64:96, :], xds[96:128, :])
                bcpt = wk.tile([96, NT], bf, tag="bcpt", name=f"bcpt{t}",
                               bufs=2)
                nc.vector.tensor_tensor(bcpt[64:96, :], xds[64:96, :],
                                        cvt[64:96, :], OP.mult)
                bcm_ps = ps_xd.tile([128, NT], f32, tag="xd",
                                    name=f"bcm_ps{t}")
                nc.tensor.matmul(bcm_ps[:], wt["combT"][64:96, :],
                                 bcpt[64:96, :])
                bcm = wk.tile([128, NT], bf, tag="bcm", name=f"bcm{t}", bufs=2)
                nc.scalar.copy(bcm[:], bcm_ps[:])

                # dt chunks -> softplus (silu + tanh correction) -> *bc
                pj = []
                for j in range(6):
                    dtp = ps_q.tile([128, NT], f32, tag="q", name=f"dt{j}_{t}")
                    nc.tensor.matmul(dtp[:],
                                     wt["wdtT"][:, 128 * j:128 * (j + 1)],
                                     xds[0:48, :])
                    sp = wk.tile([128, NT], bf, tag="sp", name=f"sp{j}_{t}",
                                 bufs=2)
                    nc.scalar.activation(sp[:], dtp[:], AF.Silu,
                                         bias=pc(f"dtb{j}"))
                    th = wk.tile([128, NT], bf, tag="th", name=f"th{j}_{t}",
                                 bufs=2)
                    nc.scalar.activation(th[:], dtp[:], AF.Tanh,
                                         bias=pc(f"dtbc{j}"), scale=SP_C)
                    spc = wk.tile([128, NT], bf, tag="spc", name=f"spc{j}_{t}",
                                  bufs=2)
                    nc.vector._custom_dve(spc_op, out=spc[:], in0=sp[:],
                                          in1=th[:], s0=SP_A)
                    pp = wk.tile([128, NT], bf, tag=f"pj{j % 3}",
                                 name=f"pj{j}_{t}", bufs=2)
                    if j % 2 == 0:
                        nc.vector.tensor_tensor(pp[:], spc[:], bcm[:], OP.mult)
                    else:
                        nc.gpsimd.tensor_mul(pp[:], spc[:], bcm[:])
                    pj.append(pp)

                gA = ps_xd.tile([128, NT], f32, tag="xd", name=f"gA{t}")
                for j in range(4):
                    nc.tensor.matmul(gA[:],
                                     wt["selsA"][:, 128 * j:128 * (j + 1)],
                                     pj[j][:], start=(j == 0), stop=(j == 3))
                gB = ps_xd.tile([64, NT], f32, tag="xd", name=f"gB{t}")
                for j in range(2):
                    nc.tensor.matmul(gB[:],
                                     wt["selsB"][:, 64 * j:64 * (j + 1)],
                                     pj[4 + j][:], start=(j == 0),
                                     stop=(j == 1))

                # y = (gain + Dsum) * xss ; LN2 stats
                yA = wk.tile([128, NT], bf, tag="yA", name=f"yA{t}", bufs=9)
                yB = wk.tile([64, NT], bf, tag="yB", name=f"yB{t}", bufs=9)
                nc.vector.scalar_tensor_tensor(yA[:], gA[:], pc("DsA"),
                                               xssA[0:128, :], OP.add, OP.mult)
                nc.vector.scalar_tensor_tensor(yB[:], gB[:], pc("DsB", 64),
                                               xssB[0:64, :], OP.add, OP.mult)
                yas[t], ybs[t] = yA, yB
                yqA = wk.tile([128, NT], bf, tag="yqA", name=f"yqA{t}", bufs=2)
                yqB = wk.tile([64, NT], bf, tag="yqB", name=f"yqB{t}", bufs=2)
                nc.scalar.activation(yqA[:], yA[:], AF.Square)
                nc.scalar.activation(yqB[:], yB[:], AF.Square)
                stp2 = ps_st.tile([33, NT], f32, tag="st", name=f"st2_{t}")
                nc.tensor.matmul(stp2[0:1, :], wt["onesb_lo"][:], yA[:],
                                 start=True, stop=False)
                nc.tensor.matmul(stp2[0:1, :], wt["onesb_hi"][:], yB[:],
                                 start=False, stop=True)
                nc.tensor.matmul(stp2[32:33, :], wt["onesb_lo"][:], yqA[:],
                                 start=True, stop=False)
                nc.tensor.matmul(stp2[32:33, :], wt["onesb_hi"][:], yqB[:],
                                 start=False, stop=True)
                ste2 = wk.tile([33, NT], f32, tag="ste", name=f"ste2_{t}",
                               bufs=2)
                nc.scalar.copy(ste2[:], stp2[:])
                nc.scalar.dma_start(batch2[i:i + 1, 0:NT], ste2[0:1, :])
                nc.scalar.dma_start(batch2[i:i + 1, NT:2 * NT],
                                    ste2[32:33, :])

            def phaseD(g, batch2):
                nrm2 = batched_rsqrt(batch2, GRP, SEED2, IT2, "2", f"2_{g}")
                nc.gpsimd.dma_start(nrm2d[g * GRP:(g + 1) * GRP, :], nrm2[:])

            def phaseE(t):
                ts = slice(t * NT, (t + 1) * NT)
                xr = wk.tile([128, NT], f32r, tag="xl", name=f"xr{t}", bufs=6)
                xrh = wk.tile([64, NT], f32r, tag="xh", name=f"xrh{t}",
                              bufs=6)
                nc.sync.dma_start(xr[:], xin[0:128, ts])
                nc.sync.dma_start(xrh[:], xin[128:192, ts])
                rb2 = wk.tile([128, NT], bf, tag="rb2", name=f"rb2_{t}",
                              bufs=2)
                mb2 = wk.tile([128, NT], bf, tag="mb2", name=f"mb2_{t}",
                              bufs=2)
                nc.gpsimd.dma_start(
                    rb2[:], nrm2d[t:t + 1, 0:NT].broadcast_to([128, NT]))
                nc.gpsimd.dma_start(
                    mb2[:], nrm2d[t:t + 1, NT:2 * NT].broadcast_to([128, NT]))

                # w = (mb2 - obp) - y*rstd2 ; v = w*z  (sign folded in Wsp)
                tA = wk.tile([128, NT], bf, tag="tA", name=f"tA{t}", bufs=2)
                tB = wk.tile([64, NT], bf, tag="tB", name=f"tB{t}", bufs=2)
                nc.vector.tensor_tensor(tA[:], yas[t][:], rb2[:], OP.mult)
                nc.gpsimd.tensor_mul(tB[:], ybs[t][:], rb2[0:64, :])
                wA = wk.tile([128, NT], bf, tag="wA", name=f"wA{t}", bufs=2)
                wB = wk.tile([64, NT], bf, tag="wB", name=f"wB{t}", bufs=2)
                nc.vector.scalar_tensor_tensor(wA[:], mb2[:], pc("obpA"),
                                               tA[:], OP.subtract, OP.subtract)
                nc.vector.scalar_tensor_tensor(wB[:], mb2[0:64, :],
                                               pc("obpB", 64), tB[:],
                                               OP.subtract, OP.subtract)
                vA = wk.tile([128, NT], bf, tag="vA", name=f"vA{t}", bufs=2)
                vB = wk.tile([64, NT], bf, tag="vB", name=f"vB{t}", bufs=2)
                nc.vector.tensor_tensor(vA[:], wA[:], zas[t][0:128, :],
                                        OP.mult)
                nc.gpsimd.tensor_mul(vB[:], wB[:], zbs[t][0:64, :])

                # s = (-Wsp') @ v
                sA = ps_mm2.tile([128, NT], f32, tag="mm2", name=f"sA{t}")
                sB = ps_mm2.tile([64, NT], f32, tag="mm2", name=f"sB{t}")
                nc.tensor.matmul(sA[:], wt["wspT_lo"][:, 0:128], vA[:],
                                 start=True, stop=False)
                nc.tensor.matmul(sA[:], wt["wspT_hi"][:, 0:128], vB[:],
                                 start=False, stop=True)
                nc.tensor.matmul(sB[:], wt["wspT_lo"][:, 128:192], vA[:],
                                 start=True, stop=False)
                nc.tensor.matmul(sB[:], wt["wspT_hi"][:, 128:192], vB[:],
                                 start=False, stop=True)

                gsA = wk.tile([128, NT], bf, tag="gsA", name=f"gsA{t}", bufs=2)
                gsB = wk.tile([64, NT], bf, tag="gsB", name=f"gsB{t}", bufs=2)
                nc.vector.tensor_tensor(gsA[:], sA[:], sx2as[t][0:128, :],
                                        OP.mult)
                nc.vector.tensor_tensor(gsB[:], sB[:], sx2bs[t][0:64, :],
                                        OP.mult)

                oA = ps_mm2.tile([128, NT], f32, tag="mm2", name=f"oA{t}")
                oB = ps_mm2.tile([64, NT], f32, tag="mm2", name=f"oB{t}")
                nc.tensor.matmul(oA[:], wt["woT_lo"][:, 0:128], gsA[:],
                                 start=True, stop=False)
                nc.tensor.matmul(oA[:], wt["woT_hi"][:, 0:128], gsB[:],
                                 start=False, stop=True)
                nc.tensor.matmul(oB[:], wt["woT_lo"][:, 128:192], gsA[:],
                                 start=True, stop=False)
                nc.tensor.matmul(oB[:], wt["woT_hi"][:, 128:192], gsB[:],
                                 start=False, stop=True)

                outA = wk.tile([128, NT], f32, tag="outA", name=f"outA{t}",
                               bufs=2)
                outB = wk.tile([64, NT], f32, tag="outB", name=f"outB{t}",
                               bufs=2)
                nc.vector.tensor_tensor(outA[:], oA[:], xr[:].bitcast(f32),
                                        OP.add)
                nc.vector.tensor_tensor(outB[:], oB[:], xrh[:].bitcast(f32),
                                        OP.add)
                nc.sync.dma_start(yout[0:128, ts], outA[:])
                nc.sync.dma_start(yout[128:192, ts], outB[:])

            batches = {}
            for g in range(NGRP):
                batch2 = wk.tile([GRP, 2 * NT], f32, tag="batch2",
                                 name=f"batch2_{g}", bufs=2)
                batches[g] = batch2
                for i in range(GRP):
                    t = g * GRP + i
                    if t + LN1_LEAD < NTILES:
                        ln1_tile(t + LN1_LEAD)
                    phaseC(t, i, batch2)
                if g > 0:
                    for i in range(GRP):
                        phaseE((g - 1) * GRP + i)
                phaseD(g, batch2)
            for i in range(GRP):
                phaseE((NGRP - 1) * GRP + i)

    nc.compile()
    return nc


def _get_program():
    if "nc" not in _CACHE:
        _CACHE["nc"] = _build_program()
    return _CACHE["nc"]


def _shard(x):
    """x (B,C,H,W) -> list of 8 (C, NPIX) channel-major slices"""
    outs = []
    for i in range(NCORES):
        b, h0 = i // 2, (i % 2) * (H // 2)
        outs.append(np.ascontiguousarray(
            x[b, :, h0:h0 + H // 2, :].reshape(C, NPIX)))
    return outs


def _shard_pm(shards):
    """channel-major shards -> pixel-major bf16 (NPIX//128, 128, C)"""
    import ml_dtypes
    outs = []
    for s in shards:
        pm = np.ascontiguousarray(s.T).astype(ml_dtypes.bfloat16)
        outs.append(pm.reshape(NPIX // 128, 128, C))
    return outs


def _unshard(parts, dtype):
    out = np.empty((B, C, H, W), dtype)
    for i in range(NCORES):
        b, h0 = i // 2, (i % 2) * (H // 2)
        out[b, :, h0:h0 + H // 2, :] = parts[i].reshape(C, H // 2, W)
    return out


def run(inputs, trace=False):
    from concourse.bass_utils import run_bass_kernel_spmd
    nc = _get_program()
    wts, _ = _fold_weights(**{k: np.asarray(v) for k, v in inputs.items()
                              if k != "x"})
    shards = _shard(np.asarray(inputs["x"], np.float32))
    pms = _shard_pm(shards)
    in_maps = [dict(wts, xin=s, xpm=p) for s, p in zip(shards, pms)]
    res = run_bass_kernel_spmd(nc, in_maps, core_ids=list(range(NCORES)),
                               trace=trace)
    out = _unshard([r["yout"] for r in res.results], np.float32)
    return out, res


def kernel(**inputs):
    out, _ = run(inputs, trace=False)
    return out


if __name__ == "__main__":
    print("building program...")
    nc = _get_program()
    print("program built")


# revision 31
# speedup vs baseline: 1.0025x; 1.0025x over previous
"""Trainium2 Bass kernel for nn_ChannelMambaBlock (v2).

Math (per pixel, channel vector x of size C=192):
  xn  = LN(x)*nw + nb
  p   = w_in @ xn              ; x1, x2 = p[:C], p[C:]
  u   = ssm_in @ x1            ; xss, z = silu(u[:C]), silu(u[C:])
  xd  = x_proj @ xss (K dirs)  ; dtr, Bv, Cv
  dt  = softplus(dt_w @ dtr + dt_b)
  bc_k = sum_s Bv*Cv ; gain = sum_k bc_k*dt_k + sum_k D_k
  y   = LN2(xss*gain)*ow + ob ; y *= z
  s   = ssm_out @ y ; o = w_out @ (s * silu(x2)) ; out = x + o

v2 kernel strategy (channel-major [C, pixels], 8-way data parallel,
16 tiles of 512 pixels per core):
  - ONE ACT table (silu_and_others: Silu/Tanh/Square/Copy). Every
    activation is fused: silu(psum+bias) evacuates the big-matmul psum
    chunks directly; softplus(x) = silu(x) + a*(1-tanh(c*x)^2) with
    fitted a,c (rmse 2e-4); rsqrt via Newton iterations on DVE
    (linear first step folds the constant seed), no Exp/Ln anywhere.
  - LN stats: PE ones-matmuls; Sum(x) from f32r x (no bf16 copy),
    Sum(x^2) from ACT Square tiles. Per-tile [2,512] psum stats are
    evacuated and DMA-stacked into [4,1024] group tiles so the scalar
    chain (mu/var/rsqrt/mu*rstd) runs once per 4 tiles.
  - xd in ONE 112-row matmul [dtr48;Bv32;Cv32]; Cv realigned onto Bv's
    partitions by an SBUF->SBUF DMA (DMA moves across partitions; DVE
    cannot).
  - dt block-diag matmul in c-major row order (row c*4+k); gain k-sum
    via 0/1 selector matmuls; bc broadcast to the c*4+k pattern with
    one selector matmul (as baseline).
  - out_norm shift handled with sign trick: w = (mb2-obp)-y*rstd2,
    v = w*z, s = (-Wsp')@v.
"""

import numpy as np

C = 192
K = 4
DT = 12
DS = 8
B, H, W = 4, 128, 128
EPS = 1e-5
NCORES = 8
NPIX = B * H * W // NCORES      # 8192 per core
NT = 512                        # pixels per tile
NTILES = NPIX // NT             # 16
GRP = 4                         # tiles per LN-scalar batch
NGRP = NTILES // GRP

# M-chunks of the big fused matmul [u(384); x2(192)]:
QCH = [(0, 128), (128, 192), (192, 320), (320, 384), (384, 512), (512, 576)]

# softplus(x) ~= silu(x) + SP_A*(1 - tanh(SP_C*x)^2)
SP_A = 0.6930162
SP_C = 0.4230186
# rsqrt Newton: y1 = 1.5*s - 0.5*s^3*v (linear in v), then iterate
SEED1, IT1 = 0.82, 3
SEED2, IT2 = 0.10, 11

NPCOL = 22

_CACHE = {}


def _fold_weights(norm_w, norm_b, w_in, ssm_in_w, x_proj_w, dt_w, dt_b,
                  A_logs, Ds, out_norm_w, out_norm_b, ssm_out_w, w_out):
    f8 = np.float64
    nw, nb = norm_w.astype(f8), norm_b.astype(f8)
    w_in = w_in.astype(f8)
    ssm = ssm_in_w.astype(f8)
    W1 = w_in * nw[None, :]
    b1 = w_in @ nb
    W_u = ssm @ W1[:C]                      # (2C, C)
    b_u = ssm @ b1[:C]
    W_big = np.concatenate([W_u, W1[C:]], 0)   # (576, C)
    b_big = np.concatenate([b_u, b1[C:]], 0)   # (576,)
    # xd rows: [dtr (48, k-major) ; pad (16) ; Bv (32) ; Cv (32)] = 128
    Xp = np.concatenate([
        x_proj_w[:, :DT].reshape(K * DT, C),
        np.zeros((16, C)),
        x_proj_w[:, DT:DT + DS].reshape(K * DS, C),
        x_proj_w[:, DT + DS:].reshape(K * DS, C)], 0).astype(f8)   # (128, C)
    # dt block: out row = c*4 + k, in col = k*12 + r
    Wdt = np.zeros((C * K, K * DT))
    for k in range(K):
        Wdt[np.arange(C) * K + k, k * DT:(k + 1) * DT] = dt_w[k].astype(f8)
    dtb_s = dt_b.astype(f8).T.reshape(C * K)  # row c*4+k
    Dsum = Ds.astype(f8).reshape(K, C).sum(0)
    ow, ob = out_norm_w.astype(f8), out_norm_b.astype(f8)
    Wsp_neg = -(ssm_out_w.astype(f8) * ow[None, :])
    obp = ob / ow
    Wo = w_out.astype(f8)

    # bc pattern: bcm[m] = bc[m%4]; bcpt rows at partitions 64:96
    combT = np.zeros((96, 128))
    for k in range(K):
        rows = 64 + k * DS + np.arange(DS)
        combT[np.ix_(rows, np.arange(128)[np.arange(128) % 4 == k])] = 1.0
    # gain selectors: chunk j of dt rows [128j,128j+128) covers c in
    # [32j, 32j+32): gain_sel_j[p, 32j + p//4] = 1
    selsA = []
    for j in range(4):
        S = np.zeros((128, 128))
        S[np.arange(128), 32 * j + np.arange(128) // 4] = 1.0
        selsA.append(S)
    selsB = []
    for j in range(2):
        S = np.zeros((128, 64))
        S[np.arange(128), 32 * j + np.arange(128) // 4] = 1.0
        selsB.append(S)

    def f32(a):
        return np.ascontiguousarray(np.asarray(a, np.float32))

    import ml_dtypes

    def bf(a):
        return np.ascontiguousarray(np.asarray(a).astype(ml_dtypes.bfloat16))

    wts = {
        "wbigT": bf(W_big.T),               # (192, 576)
        "xpT": bf(Xp.T),                    # (192, 128)
        "wdtT": bf(Wdt.T),                  # (48, 768)
        "combT": bf(combT),                 # (96, 128)
        "selsA": bf(np.concatenate(selsA, 1)),   # (128, 512)
        "selsB": bf(np.concatenate(selsB, 1)),   # (128, 128)
        "wspT": bf(Wsp_neg.T),              # (192, 192)  (negated!)
        "woT": bf(Wo.T),                    # (192, 192)
        "onescb": bf(np.ones((C, 1))),      # (192, 1) stats lhsT (bf16)
        "ident": f32(np.eye(128)),          # transpose rhs
    }
    cols = []

    def col(v):
        v = np.asarray(v, np.float64).reshape(-1)
        c = np.zeros(128)
        c[:v.size] = v
        cols.append(c)
        return len(cols) - 1

    ci = {}
    for m, (r0, r1) in enumerate(QCH):
        ci[f"b{m}"] = col(b_big[r0:r1])
    for j in range(6):
        ci[f"dtb{j}"] = col(dtb_s[128 * j:128 * (j + 1)])
        ci[f"dtbc{j}"] = col(SP_C * dtb_s[128 * j:128 * (j + 1)])
    ci["DsA"] = col(Dsum[:128]); ci["DsB"] = col(Dsum[128:])
    ci["obpA"] = col(obp[:128]); ci["obpB"] = col(obp[128:])
    assert len(cols) == NPCOL, len(cols)
    wts["pcol"] = f32(np.stack(cols, 1))     # (128, NPCOL)
    return wts, ci


def _register_customs():
    """Custom DVE ops:
      VAR_EPS_ANT:  out = in0*s0 + s1 - in1^2           (var from sums)
      NEWTON_RS_ANT: out = in1*(s0 - s1*in0*in1^2)      (rsqrt Newton step)
      SPCOMB_ANT:   out = in0 + s0 - s0*in1^2           (softplus combine)
    """
    import concourse.dve_ops as do
    from concourse.dve_spec import Spec, Src0, Src1, C0, C1, sq, lower
    from concourse.dve_uop import DveOpSpec
    if hasattr(do, "VAR_EPS_ANT"):
        return do.VAR_EPS_ANT, do.NEWTON_RS_ANT, do.SPCOMB_ANT

    def mk(name, body, ref):
        spec = Spec(body=body, reference=ref)
        opcode = do._CUSTOM_DVE_ROW_BASE + len(do.OPS)
        shas = {}
        for ver in ("v3", "v4"):
            s = DveOpSpec(name=name, opcode=opcode,
                          uops=lower(spec, ver=ver), rd1_en=True)
            shas[ver] = s.sha(ver)
        op = do.DveOp(name, spec, subdim=False, uops_sha=shas)
        do.OPS.append(op)
        do._SUB_OPCODE_FOR_NAME[op.name] = opcode
        do.CUSTOM_DVE_SPECS[op.name] = spec
        setattr(do, name, op)
        return op

    var_op = mk("VAR_EPS_ANT",
                Src0 * C0 + C1 - sq(Src1),
                lambda in0, in1, s0, s1, imm2:
                (in0.astype(np.float32) * s0 + s1 - in1 * in1))
    newt = mk("NEWTON_RS_ANT",
              Src1 * (C0 - C1 * Src0 * sq(Src1)),
              lambda in0, in1, s0, s1, imm2:
              (in1.astype(np.float32)
               * (s0 - s1 * in0 * in1 * in1)))
    spc = mk("SPCOMB_ANT",
             Src0 + C0 - C0 * sq(Src1),
             lambda in0, in1, s0, s1, imm2:
             (in0.astype(np.float32) + s0 - s0 * in1 * in1))
    return var_op, newt, spc


def _patch_act_tables():
    """Pin every activation we use (Silu/Tanh/Square/Copy/Identity) to the
    single silu_and_others table so one hoisted ACT table load serves the
    whole kernel."""
    import concourse.mybir as mybir
    import concourse.bacc as bacc_mod
    import concourse.hw_specs as hw
    if getattr(bacc_mod, "_ant_act_tables_patched", False):
        return
    orig = hw.get_activation_tables
    AF = mybir.ActivationFunctionType
    keep = "silu_and_others"
    mine = {AF.Silu, AF.Tanh, AF.Square, AF.Copy, AF.Identity}

    def patched(arch):
        t = dict(orig(arch))
        for name in list(t.keys()):
            if name != keep:
                t[name] = t[name] - mine
        return t

    bacc_mod.get_activation_tables = patched
    bacc_mod._ant_act_tables_patched = True


def _build_program():
    import concourse.mybir as mybir
    from concourse.bacc import Bacc
    from concourse.tile import TileContext
    from concourse.alu_op_type import AluOpType

    _patch_act_tables()
    var_op, newt_op, spc_op = _register_customs()

    AF = mybir.ActivationFunctionType
    OP = AluOpType
    f32 = mybir.dt.float32
    f32r = mybir.dt.float32r
    bf = mybir.dt.bfloat16
    nc = Bacc()

    xin = nc.dram_tensor("xin", (C, NPIX), f32r, kind="ExternalInput")
    yout = nc.dram_tensor("yout", (C, NPIX), f32, kind="ExternalOutput")
    dw = {}
    for name, shape in [("wbigT", (C, 576)), ("xpT", (C, 128)),
                        ("wdtT", (48, 768)), ("combT", (96, 128)),
                        ("selsA", (128, 512)), ("selsB", (128, 128)),
                        ("wspT", (C, 192)), ("woT", (C, 192))]:
        dw[name] = nc.dram_tensor(name, shape, bf, kind="ExternalInput")
    dw["onescb"] = nc.dram_tensor("onescb", (C, 1), bf, kind="ExternalInput")
    dw["ident"] = nc.dram_tensor("ident", (128, 128), f32, kind="ExternalInput")
    xpm = nc.dram_tensor("xpm", (NPIX // 128, 128, C), bf,
                         kind="ExternalInput")
    dw["pcol"] = nc.dram_tensor("pcol", (128, NPCOL), f32, kind="ExternalInput")
    nrm1d = nc.dram_tensor("nrm1d", (NTILES, 2 * NT), bf, kind="Internal")
    nrm2d = nc.dram_tensor("nrm2d", (NTILES, 2 * NT), bf, kind="Internal")

    with TileContext(nc) as tc:
        with (
            tc.tile_pool(name="wpool", bufs=1) as wp,
            tc.tile_pool(name="work", bufs=1) as wk,
            tc.tile_pool(name="ps_q", bufs=3, space="PSUM") as ps_q,     # q + dt
            tc.tile_pool(name="ps_mm2", bufs=2, space="PSUM") as ps_mm2,  # gain/s/o
            tc.tile_pool(name="ps_xd", bufs=2, space="PSUM") as ps_xd,   # xd/bcm
            tc.tile_pool(name="ps_st", bufs=1, space="PSUM") as ps_st,   # stats
        ):
            # ---- load weights once ----
            wt = {}
            for name, shape in [("wbigT_lo", (128, 576)), ("wbigT_hi", (64, 576)),
                                ("xpT_lo", (128, 128)), ("xpT_hi", (64, 128)),
                                ("wdtT", (48, 768)), ("combT", (96, 128)),
                                ("selsA", (128, 512)), ("selsB", (128, 128)),
                                ("wspT_lo", (128, 192)), ("wspT_hi", (64, 192)),
                                ("woT_lo", (128, 192)), ("woT_hi", (64, 192)),
                                ("onesb_lo", (128, 1)), ("onesb_hi", (64, 1))]:
                wt[name] = wp.tile(list(shape), bf, tag=name, name=name)
            wt["ident"] = wp.tile([128, 128], f32, tag="ident", name="ident")
            wt["pcol"] = wp.tile([128, NPCOL], f32, tag="pcol", name="pcol")
            for nm, src in [("wbigT_lo", dw["wbigT"][0:128, :]),
                            ("wbigT_hi", dw["wbigT"][128:192, :]),
                            ("xpT_lo", dw["xpT"][0:128, :]),
                            ("xpT_hi", dw["xpT"][128:192, :]),
                            ("wdtT", dw["wdtT"][:]),
                            ("combT", dw["combT"][:]),
                            ("selsA", dw["selsA"][:]),
                            ("selsB", dw["selsB"][:]),
                            ("wspT_lo", dw["wspT"][0:128, :]),
                            ("wspT_hi", dw["wspT"][128:192, :]),
                            ("woT_lo", dw["woT"][0:128, :]),
                            ("woT_hi", dw["woT"][128:192, :]),
                            ("ident", dw["ident"][:]),
                            ("onesb_lo", dw["onescb"][0:128, :]),
                            ("onesb_hi", dw["onescb"][128:192, :]),
                            ("pcol", dw["pcol"][:])]:
                nc.sync.dma_start(wt[nm][:], src)

            PC = {}
            idx = 0
            for m in range(6):
                PC[f"b{m}"] = idx; idx += 1
            for j in range(6):
                PC[f"dtb{j}"] = idx; idx += 1
                PC[f"dtbc{j}"] = idx; idx += 1
            for nm in ["DsA", "DsB", "obpA", "obpB"]:
                PC[nm] = idx; idx += 1

            def pc(name, rows=128):
                return wt["pcol"][0:rows, PC[name]:PC[name] + 1]

            def batched_rsqrt(batch, rows, seed, iters, tag, uid,
                              r0=0, r1=None, nrm=None):
                """batch [rows,1024] f32 (cols 0:512 sum(x), 512:1024
                sum(x^2)); processes row slice [r0:r1]. Returns nrm
                [rows,1024] bf (cols 0:512 rstd, 512:1024 mu*rstd)."""
                if r1 is None:
                    r1 = rows
                rs = slice(r0, r1)
                mu = wk.tile([rows, NT], f32, tag=f"mu{tag}",
                             name=f"mu{uid}", bufs=1)
                nc.vector.tensor_scalar_mul(mu[rs, :], batch[rs, 0:NT],
                                            1.0 / C)
                var = wk.tile([rows, NT], f32, tag=f"var{tag}",
                              name=f"var{uid}", bufs=1)
                nc.vector._custom_dve(var_op, out=var[rs, :],
                                      in0=batch[rs, NT:2 * NT], in1=mu[rs, :],
                                      s0=1.0 / C, s1=EPS)
                y0 = wk.tile([rows, NT], f32, tag=f"nt{tag}",
                             name=f"y0{uid}", bufs=2)
                nc.vector.tensor_scalar(y0[rs, :], var[rs, :],
                                        -0.5 * seed ** 3,
                                        1.5 * seed, OP.mult, OP.add)
                cur = y0
                if nrm is None:
                    nrm = wk.tile([rows, 2 * NT], bf, tag=f"nrm{tag}",
                                  name=f"nrm{uid}", bufs=2)
                for i in range(iters):
                    last = (i == iters - 1)
                    if last:
                        nc.vector._custom_dve(newt_op, out=nrm[rs, 0:NT],
                                              in0=var[rs, :], in1=cur[rs, :],
                                              s0=1.5, s1=0.5)
                    else:
                        yn = wk.tile([rows, NT], f32, tag=f"nt{tag}",
                                     name=f"y{i + 1}{uid}", bufs=2)
                        nc.vector._custom_dve(newt_op, out=yn[rs, :],
                                              in0=var[rs, :], in1=cur[rs, :],
                                              s0=1.5, s1=0.5)
                        cur = yn
                nc.vector.tensor_tensor(nrm[rs, NT:2 * NT], mu[rs, :],
                                        nrm[rs, 0:NT], OP.mult)
                return nrm

            sx2as, sx2bs, zas, zbs, yas, ybs = {}, {}, {}, {}, {}, {}

            # (LN1 is computed per tile inside phaseC via pixel-major
            # DVE reduces; no prologue needed.)

            # ================= MAIN LOOP =================
            def ln1_tile(t):
                # ---- LN1 stats from pixel-major x (DVE/GPS reduces) ----
                xp = wk.tile([128, 4 * C], bf, tag="xp", name=f"xp{t}", bufs=3)
                nc.sync.dma_start(
                    xp[:], xpm[4 * t:4 * t + 4, :, :].rearrange(
                        "j p c -> p j c"))
                xp3 = xp[:].rearrange("p (j c) -> p j c", j=4)
                sqp = wk.tile([128, 4 * C], bf, tag="sqp", name=f"sqp{t}",
                              bufs=2)
                nc.gpsimd.tensor_mul(sqp[:], xp[:], xp[:])
                ssum = wk.tile([128, 4], f32, tag="ssum", name=f"ssum{t}",
                               bufs=2)
                nc.vector.tensor_reduce(ssum[:].unsqueeze(2),
                                        xp3, mybir.AxisListType.X, OP.add)
                qsum = wk.tile([128, 4], f32, tag="qsum", name=f"qsum{t}",
                               bufs=2)
                nc.vector.tensor_reduce(
                    qsum[:].unsqueeze(2),
                    sqp[:].rearrange("p (j c) -> p j c", j=4),
                    mybir.AxisListType.X, OP.add)
                mu = wk.tile([128, 4], f32, tag="mus", name=f"mus{t}", bufs=2)
                nc.vector.tensor_scalar_mul(mu[:], ssum[:], 1.0 / C)
                var = wk.tile([128, 4], f32, tag="vars", name=f"vars{t}",
                              bufs=2)
                nc.vector._custom_dve(var_op, out=var[:], in0=qsum[:],
                                      in1=mu[:], s0=1.0 / C, s1=EPS)
                cur = wk.tile([128, 4], f32, tag="nts", name=f"y0s{t}", bufs=2)
                nc.vector.tensor_scalar(cur[:], var[:], -0.5 * SEED1 ** 3,
                                        1.5 * SEED1, OP.mult, OP.add)
                pk = wk.tile([128, 8], f32, tag="pks", name=f"pks{t}", bufs=2)
                for it in range(IT1):
                    dst = (pk[:, 0:4] if it == IT1 - 1 else
                           wk.tile([128, 4], f32, tag="nts",
                                   name=f"y{it + 1}s{t}", bufs=2)[:])
                    nc.vector._custom_dve(newt_op, out=dst, in0=var[:],
                                          in1=cur[:], s0=1.5, s1=0.5)
                    cur = dst
                nc.vector.tensor_tensor(pk[:, 4:8], mu[:], pk[:, 0:4],
                                        OP.mult)
                ptr = ps_st.tile([8, 128], f32, tag="st", name=f"ptr{t}")
                nc.tensor.transpose(ptr[:], pk[:], wt["ident"][:])
                ntr = wk.tile([8, 128], bf, tag="ntr", name=f"ntr{t}", bufs=2)
                nc.scalar.copy(ntr[:], ptr[:])
                nc.scalar.dma_start(
                    nrm1d[t:t + 1, 0:NT].rearrange("o (j p) -> o j p", j=4),
                    ntr[0:4, :])
                nc.scalar.dma_start(
                    nrm1d[t:t + 1, NT:2 * NT].rearrange("o (j p) -> o j p",
                                                        j=4),
                    ntr[4:8, :])


            for t in range(NTILES):
                ln1_tile(t)

            def phaseC(t, i, batch2):
                ts = slice(t * NT, (t + 1) * NT)
                xc = wk.tile([128, NT], f32r, tag="xl", name=f"xc{t}", bufs=6)
                xch = wk.tile([64, NT], f32r, tag="xh", name=f"xch{t}",
                              bufs=6)
                nc.sync.dma_start(xc[:], xin[0:128, ts])
                nc.sync.dma_start(xch[:], xin[128:192, ts])

                rb = wk.tile([128, NT], bf, tag="rb", name=f"rb{t}", bufs=2)
                mb = wk.tile([128, NT], bf, tag="mb", name=f"mb{t}", bufs=2)
                nc.gpsimd.dma_start(
                    rb[:], nrm1d[t:t + 1, 0:NT].broadcast_to([128, NT]))
                nc.gpsimd.dma_start(
                    mb[:], nrm1d[t:t + 1, NT:2 * NT].broadcast_to([128, NT]))

                # x_hat = x*rstd - mu*rstd
                xt = wk.tile([128, NT], bf, tag="xt", name=f"xt{t}", bufs=2)
                xhl = wk.tile([128, NT], bf, tag="xhl", name=f"xhl{t}", bufs=2)
                xhh = wk.tile([64, NT], bf, tag="xhh", name=f"xhh{t}", bufs=2)
                nc.vector.tensor_tensor(xt[:], xc[:].bitcast(f32), rb[:],
                                        OP.mult)
                nc.vector.tensor_tensor(xhl[:], xt[:], mb[:], OP.subtract)
                nc.vector.tensor_tensor(xt[0:64, :], xch[:].bitcast(f32),
                                        rb[0:64, :], OP.mult)
                nc.vector.tensor_tensor(xhh[:], xt[0:64, :], mb[0:64, :],
                                        OP.subtract)

                # big matmul; silu fused into psum evac
                souts = []
                for m, (r0, r1) in enumerate(QCH):
                    rows = r1 - r0
                    q = ps_q.tile([rows, NT], f32, tag="q", name=f"q{m}_{t}")
                    nc.tensor.matmul(q[:], wt["wbigT_lo"][:, r0:r1], xhl[:],
                                     start=True, stop=False)
                    nc.tensor.matmul(q[:], wt["wbigT_hi"][:, r0:r1], xhh[:],
                                     start=False, stop=True)
                    so = wk.tile([128, NT], bf, tag=f"sil{m}",
                                 name=f"sil{m}_{t}",
                                 bufs=2 if m in (0, 1) else 9)
                    nc.scalar.activation(so[0:rows, :], q[:], AF.Silu,
                                         bias=pc(f"b{m}", rows))
                    souts.append(so)
                xssA, xssB = souts[0], souts[1]
                zas[t], zbs[t] = souts[2], souts[3]
                sx2as[t], sx2bs[t] = souts[4], souts[5]

                # xd = [dtr;pad;Bv;Cv] @ xss  (one 128-row matmul)
                xdp = ps_xd.tile([128, NT], f32, tag="xd", name=f"xd{t}")
                nc.tensor.matmul(xdp[:], wt["xpT_lo"][:], xssA[0:128, :],
                                 start=True, stop=False)
                nc.tensor.matmul(xdp[:], wt["xpT_hi"][:], xssB[0:64, :],
                                 start=False, stop=True)
                xds = wk.tile([128, NT], bf, tag="xds", name=f"xds{t}", bufs=2)
                nc.scalar.copy(xds[:], xdp[:])
                # realign Cv (rows 96:128) onto Bv's partitions 64:96
                cvt = wk.tile([96, NT], bf, tag="cvt", name=f"cvt{t}", bufs=2)
                nc.sync.dma_start(cvt[64:96, :], xds[96:128, :])
                bcpt = wk.tile([96, NT], bf, tag="bcpt", name=f"bcpt{t}",
                               bufs=2)
                nc.vector.tensor_tensor(bcpt[64:96, :], xds[64:96, :],
                                        cvt[64:96, :], OP.mult)
                bcm_ps = ps_xd.tile([128, NT], f32, tag="xd",
                                    name=f"bcm_ps{t}")
                nc.tensor.matmul(bcm_ps[:], wt["combT"][64:96, :],
                                 bcpt[64:96, :])
                bcm = wk.tile([128, NT], bf, tag="bcm", name=f"bcm{t}", bufs=2)
                nc.scalar.copy(bcm[:], bcm_ps[:])

                # dt chunks -> softplus (silu + tanh correction) -> *bc
                pj = []
                for j in range(6):
                    dtp = ps_q.tile([128, NT], f32, tag="q", name=f"dt{j}_{t}")
                    nc.tensor.matmul(dtp[:],
                                     wt["wdtT"][:, 128 * j:128 * (j + 1)],
                                     xds[0:48, :])
                    sp = wk.tile([128, NT], bf, tag="sp", name=f"sp{j}_{t}",
                                 bufs=2)
                    nc.scalar.activation(sp[:], dtp[:], AF.Silu,
                                         bias=pc(f"dtb{j}"))
                    th = wk.tile([128, NT], bf, tag="th", name=f"th{j}_{t}",
                                 bufs=2)
                    nc.scalar.activation(th[:], dtp[:], AF.Tanh,
                                         bias=pc(f"dtbc{j}"), scale=SP_C)
                    spc = wk.tile([128, NT], bf, tag="spc", name=f"spc{j}_{t}",
                                  bufs=2)
                    nc.vector._custom_dve(spc_op, out=spc[:], in0=sp[:],
                                          in1=th[:], s0=SP_A)
                    pp = wk.tile([128, NT], bf, tag=f"pj{j % 3}",
                                 name=f"pj{j}_{t}", bufs=2)
                    if j % 2 == 0:
                        nc.vector.tensor_tensor(pp[:], spc[:], bcm[:], OP.mult)
                    else:
                        nc.gpsimd.tensor_mul(pp[:], spc[:], bcm[:])
                    pj.append(pp)

                gA = ps_xd.tile([128, NT], f32, tag="xd", name=f"gA{t}")
                for j in range(4):
                    nc.tensor.matmul(gA[:],
                                     wt["selsA"][:, 128 * j:128 * (j + 1)],
                                     pj[j][:], start=(j == 0), stop=(j == 3))
                gB = ps_xd.tile([64, NT], f32, tag="xd", name=f"gB{t}")
                for j in range(2):
                    nc.tensor.matmul(gB[:],
                                     wt["selsB"][:, 64 * j:64 * (j + 1)],
                                     pj[4 + j][:], start=(j == 0),
                                     stop=(j == 1))

                # y = (gain + Dsum) * xss ; LN2 stats
                yA = wk.tile([128, NT], bf, tag="yA", name=f"yA{t}", bufs=9)
                yB = wk.tile([64, NT], bf, tag="yB", name=f"yB{t}", bufs=9)
                nc.vector.scalar_tensor_tensor(yA[:], gA[:], pc("DsA"),
                                               xssA[0:128, :], OP.add, OP.mult)
                nc.vector.scalar_tensor_tensor(yB[:], gB[:], pc("DsB", 64),
                                               xssB[0:64, :], OP.add, OP.mult)
                yas[t], ybs[t] = yA, yB
                yqA = wk.tile([128, NT], bf, tag="yqA", name=f"yqA{t}", bufs=2)
                yqB = wk.tile([64, NT], bf, tag="yqB", name=f"yqB{t}", bufs=2)
                nc.scalar.activation(yqA[:], yA[:], AF.Square)
                nc.scalar.activation(yqB[:], yB[:], AF.Square)
                stp2 = ps_st.tile([33, NT], f32, tag="st", name=f"st2_{t}")
                nc.tensor.matmul(stp2[0:1, :], wt["onesb_lo"][:], yA[:],
                                 start=True, stop=False)
                nc.tensor.matmul(stp2[0:1, :], wt["onesb_hi"][:], yB[:],
                                 start=False, stop=True)
                nc.tensor.matmul(stp2[32:33, :], wt["onesb_lo"][:], yqA[:],
                                 start=True, stop=False)
                nc.tensor.matmul(stp2[32:33, :], wt["onesb_hi"][:], yqB[:],
                                 start=False, stop=True)
                ste2 = wk.tile([33, NT], f32, tag="ste", name=f"ste2_{t}",
                               bufs=2)
                nc.scalar.copy(ste2[:], stp2[:])
                nc.scalar.dma_start(batch2[i:i + 1, 0:NT], ste2[0:1, :])
                nc.scalar.dma_start(batch2[i:i + 1, NT:2 * NT],
                                    ste2[32:33, :])

            def phaseD(g, batch2):
                nrm2 = batched_rsqrt(batch2, GRP, SEED2, IT2, "2", f"2_{g}")
                nc.gpsimd.dma_start(nrm2d[g * GRP:(g + 1) * GRP, :], nrm2[:])

            def phaseE(t):
                ts = slice(t * NT, (t + 1) * NT)
                xr = wk.tile([128, NT], f32r, tag="xl", name=f"xr{t}", bufs=6)
                xrh = wk.tile([64, NT], f32r, tag="xh", name=f"xrh{t}",
                              bufs=6)
                nc.sync.dma_start(xr[:], xin[0:128, ts])
                nc.sync.dma_start(xrh[:], xin[128:192, ts])
                rb2 = wk.tile([128, NT], bf, tag="rb2", name=f"rb2_{t}",
                              bufs=2)
                mb2 = wk.tile([128, NT], bf, tag="mb2", name=f"mb2_{t}",
                              bufs=2)
                nc.gpsimd.dma_start(
                    rb2[:], nrm2d[t:t + 1, 0:NT].broadcast_to([128, NT]))
                nc.gpsimd.dma_start(
                    mb2[:], nrm2d[t:t + 1, NT:2 * NT].broadcast_to([128, NT]))

                # w = (mb2 - obp) - y*rstd2 ; v = w*z  (sign folded in Wsp)
                tA = wk.tile([128, NT], bf, tag="tA", name=f"tA{t}", bufs=2)
                tB = wk.tile([64, NT], bf, tag="tB", name=f"tB{t}", bufs=2)
                nc.vector.tensor_tensor(tA[:], yas[t][:], rb2[:], OP.mult)
                nc.gpsimd.tensor_mul(tB[:], ybs[t][:], rb2[0:64, :])
                wA = wk.tile([128, NT], bf, tag="wA", name=f"wA{t}", bufs=2)
                wB = wk.tile([64, NT], bf, tag="wB", name=f"wB{t}", bufs=2)
                nc.vector.scalar_tensor_tensor(wA[:], mb2[:], pc("obpA"),
                                               tA[:], OP.subtract, OP.subtract)
                nc.vector.scalar_tensor_tensor(wB[:], mb2[0:64, :],
                                               pc("obpB", 64), tB[:],
                                               OP.subtract, OP.subtract)
                vA = wk.tile([128, NT], bf, tag="vA", name=f"vA{t}", bufs=2)
                vB = wk.tile([64, NT], bf, tag="vB", name=f"vB{t}", bufs=2)
                nc.vector.tensor_tensor(vA[:], wA[:], zas[t][0:128, :],
                                        OP.mult)
                nc.gpsimd.tensor_mul(vB[:], wB[:], zbs[t][0:64, :])

                # s = (-Wsp') @ v
                sA = ps_mm2.tile([128, NT], f32, tag="mm2", name=f"sA{t}")
                sB = ps_mm2.tile([64, NT], f32, tag="mm2", name=f"sB{t}")
                nc.tensor.matmul(sA[:], wt["wspT_lo"][:, 0:128], vA[:],
                                 start=True, stop=False)
                nc.tensor.matmul(sA[:], wt["wspT_hi"][:, 0:128], vB[:],
                                 start=False, stop=True)
                nc.tensor.matmul(sB[:], wt["wspT_lo"][:, 128:192], vA[:],
                                 start=True, stop=False)
                nc.tensor.matmul(sB[:], wt["wspT_hi"][:, 128:192], vB[:],
                                 start=False, stop=True)

                gsA = wk.tile([128, NT], bf, tag="gsA", name=f"gsA{t}", bufs=2)
                gsB = wk.tile([64, NT], bf, tag="gsB", name=f"gsB{t}", bufs=2)
                nc.vector.tensor_tensor(gsA[:], sA[:], sx2as[t][0:128, :],
                                        OP.mult)
                nc.vector.tensor_tensor(gsB[:], sB[:], sx2bs[t][0:64, :],
                                        OP.mult)

                oA = ps_mm2.tile([128, NT], f32, tag="mm2", name=f"oA{t}")
                oB = ps_mm2.tile([64, NT], f32, tag="mm2", name=f"oB{t}")
                nc.tensor.matmul(oA[:], wt["woT_lo"][:, 0:128], gsA[:],
                                 start=True, stop=False)
                nc.tensor.matmul(oA[:], wt["woT_hi"][:, 0:128], gsB[:],
                                 start=False, stop=True)
                nc.tensor.matmul(oB[:], wt["woT_lo"][:, 128:192], gsA[:],
                                 start=True, stop=False)
                nc.tensor.matmul(oB[:], wt["woT_hi"][:, 128:192], gsB[:],
                                 start=False, stop=True)

                outA = wk.tile([128, NT], f32, tag="outA", name=f"outA{t}",
                               bufs=2)
                outB = wk.tile([64, NT], f32, tag="outB", name=f"outB{t}",
                               bufs=2)
                nc.vector.tensor_tensor(outA[:], oA[:], xr[:].bitcast(f32),
                                        OP.add)
                nc.vector.tensor_tensor(outB[:], oB[:], xrh[:].bitcast(f32),
                                        OP.add)
                nc.sync.dma_start(yout[0:128, ts], outA[:])
                nc.sync.dma_start(yout[128:192, ts], outB[:])

            batches = {}
            for g in range(NGRP):
                batch2 = wk.tile([GRP, 2 * NT], f32, tag="batch2",
                                 name=f"batch2_{g}", bufs=2)
                batches[g] = batch2
                for i in range(GRP):
                    phaseC(g * GRP + i, i, batch2)
                if g > 0:
                    for i in range(GRP):
                        phaseE((g - 1) * GRP + i)
                phaseD(g, batch2)
            for i in range(GRP):
                phaseE((NGRP - 1) * GRP + i)

    nc.compile()
    return nc


def _get_program():
    if "nc" not in _CACHE:
        _CACHE["nc"] = _build_program()
    return _CACHE["nc"]


def _shard(x):
    """x (B,C,H,W) -> list of 8 (C, NPIX) channel-major slices"""
    outs = []
    for i in range(NCORES):
        b, h0 = i // 2, (i % 2) * (H // 2)
        outs.append(np.ascontiguousarray(
            x[b, :, h0:h0 + H // 2, :].reshape(C, NPIX)))
    return outs


def _shard_pm(shards):
    """channel-major shards -> pixel-major bf16 (NPIX//128, 128, C)"""
    import ml_dtypes
    outs = []
    for s in shards:
        pm = np.ascontiguousarray(s.T).astype(ml_dtypes.bfloat16)
        outs.append(pm.reshape(NPIX // 128, 128, C))
    return outs


def _unshard(parts, dtype):
    out = np.empty((B, C, H, W), dtype)
    for i in range(NCORES):
        b, h0 = i // 2, (i % 2) * (H // 2)
        out[b, :, h0:h0 + H // 2, :] = parts[i].reshape(C, H // 2, W)
    return out


def run(inputs, trace=False):
    from concourse.bass_utils import run_bass_kernel_spmd
    nc = _get_program()
    wts, _ = _fold_weights(**{k: np.asarray(v) for k, v in inputs.items()
                              if k != "x"})
    shards = _shard(np.asarray(inputs["x"], np.float32))
    pms = _shard_pm(shards)
    in_maps = [dict(wts, xin=s, xpm=p) for s, p in zip(shards, pms)]
    res = run_bass_kernel_spmd(nc, in_maps, core_ids=list(range(NCORES)),
                               trace=trace)
    out = _unshard([r["yout"] for r in res.results], np.float32)
    return out, res


def kernel(**inputs):
    out, _ = run(inputs, trace=False)
    return out


if __name__ == "__main__":
    print("building program...")
    nc = _get_program()
    print("program built")


# revision 32
# speedup vs baseline: 1.0418x; 1.0392x over previous
"""Trainium2 Bass kernel for nn_ChannelMambaBlock (v2).

Math (per pixel, channel vector x of size C=192):
  xn  = LN(x)*nw + nb
  p   = w_in @ xn              ; x1, x2 = p[:C], p[C:]
  u   = ssm_in @ x1            ; xss, z = silu(u[:C]), silu(u[C:])
  xd  = x_proj @ xss (K dirs)  ; dtr, Bv, Cv
  dt  = softplus(dt_w @ dtr + dt_b)
  bc_k = sum_s Bv*Cv ; gain = sum_k bc_k*dt_k + sum_k D_k
  y   = LN2(xss*gain)*ow + ob ; y *= z
  s   = ssm_out @ y ; o = w_out @ (s * silu(x2)) ; out = x + o

v2 kernel strategy (channel-major [C, pixels], 8-way data parallel,
16 tiles of 512 pixels per core):
  - ONE ACT table (silu_and_others: Silu/Tanh/Square/Copy). Every
    activation is fused: silu(psum+bias) evacuates the big-matmul psum
    chunks directly; softplus(x) = silu(x) + a*(1-tanh(c*x)^2) with
    fitted a,c (rmse 2e-4); rsqrt via Newton iterations on DVE
    (linear first step folds the constant seed), no Exp/Ln anywhere.
  - LN stats: PE ones-matmuls; Sum(x) from f32r x (no bf16 copy),
    Sum(x^2) from ACT Square tiles. Per-tile [2,512] psum stats are
    evacuated and DMA-stacked into [4,1024] group tiles so the scalar
    chain (mu/var/rsqrt/mu*rstd) runs once per 4 tiles.
  - xd in ONE 112-row matmul [dtr48;Bv32;Cv32]; Cv realigned onto Bv's
    partitions by an SBUF->SBUF DMA (DMA moves across partitions; DVE
    cannot).
  - dt block-diag matmul in c-major row order (row c*4+k); gain k-sum
    via 0/1 selector matmuls; bc broadcast to the c*4+k pattern with
    one selector matmul (as baseline).
  - out_norm shift handled with sign trick: w = (mb2-obp)-y*rstd2,
    v = w*z, s = (-Wsp')@v.
"""

import numpy as np

C = 192
K = 4
DT = 12
DS = 8
B, H, W = 4, 128, 128
EPS = 1e-5
NCORES = 8
NPIX = B * H * W // NCORES      # 8192 per core
NT = 512                        # pixels per tile
NTILES = NPIX // NT             # 16
GRP = 4                         # tiles per LN-scalar batch
NGRP = NTILES // GRP

# M-chunks of the big fused matmul [u(384); x2(192)]:
QCH = [(0, 128), (128, 192), (192, 320), (320, 384), (384, 512), (512, 576)]

# softplus(x) ~= silu(x) + SP_A*(1 - tanh(SP_C*x)^2)
SP_A = 0.6930162
SP_C = 0.4230186
# rsqrt Newton: y1 = 1.5*s - 0.5*s^3*v (linear in v), then iterate
SEED1, IT1 = 0.82, 3
SEED2, IT2 = 0.10, 11

NPCOL = 22

_CACHE = {}


def _fold_weights(norm_w, norm_b, w_in, ssm_in_w, x_proj_w, dt_w, dt_b,
                  A_logs, Ds, out_norm_w, out_norm_b, ssm_out_w, w_out):
    f8 = np.float64
    nw, nb = norm_w.astype(f8), norm_b.astype(f8)
    w_in = w_in.astype(f8)
    ssm = ssm_in_w.astype(f8)
    W1 = w_in * nw[None, :]
    b1 = w_in @ nb
    W_u = ssm @ W1[:C]                      # (2C, C)
    b_u = ssm @ b1[:C]
    W_big = np.concatenate([W_u, W1[C:]], 0)   # (576, C)
    b_big = np.concatenate([b_u, b1[C:]], 0)   # (576,)
    # xd rows: [dtr (48, k-major) ; pad (16) ; Bv (32) ; Cv (32)] = 128
    Xp = np.concatenate([
        x_proj_w[:, :DT].reshape(K * DT, C),
        np.zeros((16, C)),
        x_proj_w[:, DT:DT + DS].reshape(K * DS, C),
        x_proj_w[:, DT + DS:].reshape(K * DS, C)], 0).astype(f8)   # (128, C)
    # dt block: out row = c*4 + k, in col = k*12 + r
    Wdt = np.zeros((C * K, K * DT))
    for k in range(K):
        Wdt[np.arange(C) * K + k, k * DT:(k + 1) * DT] = dt_w[k].astype(f8)
    dtb_s = dt_b.astype(f8).T.reshape(C * K)  # row c*4+k
    Dsum = Ds.astype(f8).reshape(K, C).sum(0)
    ow, ob = out_norm_w.astype(f8), out_norm_b.astype(f8)
    Wsp_neg = -(ssm_out_w.astype(f8) * ow[None, :])
    obp = ob / ow
    Wo = w_out.astype(f8)

    # bc pattern: bcm[m] = bc[m%4]; bcpt rows at partitions 64:96
    combT = np.zeros((96, 128))
    for k in range(K):
        rows = 64 + k * DS + np.arange(DS)
        combT[np.ix_(rows, np.arange(128)[np.arange(128) % 4 == k])] = 1.0
    # gain selectors: chunk j of dt rows [128j,128j+128) covers c in
    # [32j, 32j+32): gain_sel_j[p, 32j + p//4] = 1
    selsA = []
    for j in range(4):
        S = np.zeros((128, 128))
        S[np.arange(128), 32 * j + np.arange(128) // 4] = 1.0
        selsA.append(S)
    selsB = []
    for j in range(2):
        S = np.zeros((128, 64))
        S[np.arange(128), 32 * j + np.arange(128) // 4] = 1.0
        selsB.append(S)

    def f32(a):
        return np.ascontiguousarray(np.asarray(a, np.float32))

    import ml_dtypes

    def bf(a):
        return np.ascontiguousarray(np.asarray(a).astype(ml_dtypes.bfloat16))

    wts = {
        "wbigT": bf(W_big.T),               # (192, 576)
        "xpT": bf(Xp.T),                    # (192, 128)
        "wdtT": bf(Wdt.T),                  # (48, 768)
        "combT": bf(combT),                 # (96, 128)
        "selsA": bf(np.concatenate(selsA, 1)),   # (128, 512)
        "selsB": bf(np.concatenate(selsB, 1)),   # (128, 128)
        "wspT": bf(Wsp_neg.T),              # (192, 192)  (negated!)
        "woT": bf(Wo.T),                    # (192, 192)
        "onescb": bf(np.ones((C, 1))),      # (192, 1) stats lhsT (bf16)
        "ident": f32(np.eye(128)),          # transpose rhs
    }
    cols = []

    def col(v):
        v = np.asarray(v, np.float64).reshape(-1)
        c = np.zeros(128)
        c[:v.size] = v
        cols.append(c)
        return len(cols) - 1

    ci = {}
    for m, (r0, r1) in enumerate(QCH):
        ci[f"b{m}"] = col(b_big[r0:r1])
    for j in range(6):
        ci[f"dtb{j}"] = col(dtb_s[128 * j:128 * (j + 1)])
        ci[f"dtbc{j}"] = col(SP_C * dtb_s[128 * j:128 * (j + 1)])
    ci["DsA"] = col(Dsum[:128]); ci["DsB"] = col(Dsum[128:])
    ci["obpA"] = col(obp[:128]); ci["obpB"] = col(obp[128:])
    assert len(cols) == NPCOL, len(cols)
    wts["pcol"] = f32(np.stack(cols, 1))     # (128, NPCOL)
    return wts, ci


def _register_customs():
    """Custom DVE ops:
      VAR_EPS_ANT:  out = in0*s0 + s1 - in1^2           (var from sums)
      NEWTON_RS_ANT: out = in1*(s0 - s1*in0*in1^2)      (rsqrt Newton step)
      SPCOMB_ANT:   out = in0 + s0 - s0*in1^2           (softplus combine)
    """
    import concourse.dve_ops as do
    from concourse.dve_spec import Spec, Src0, Src1, C0, C1, sq, lower
    from concourse.dve_uop import DveOpSpec
    if hasattr(do, "VAR_EPS_ANT"):
        return do.VAR_EPS_ANT, do.NEWTON_RS_ANT, do.SPCOMB_ANT

    def mk(name, body, ref):
        spec = Spec(body=body, reference=ref)
        opcode = do._CUSTOM_DVE_ROW_BASE + len(do.OPS)
        shas = {}
        for ver in ("v3", "v4"):
            s = DveOpSpec(name=name, opcode=opcode,
                          uops=lower(spec, ver=ver), rd1_en=True)
            shas[ver] = s.sha(ver)
        op = do.DveOp(name, spec, subdim=False, uops_sha=shas)
        do.OPS.append(op)
        do._SUB_OPCODE_FOR_NAME[op.name] = opcode
        do.CUSTOM_DVE_SPECS[op.name] = spec
        setattr(do, name, op)
        return op

    var_op = mk("VAR_EPS_ANT",
                Src0 * C0 + C1 - sq(Src1),
                lambda in0, in1, s0, s1, imm2:
                (in0.astype(np.float32) * s0 + s1 - in1 * in1))
    newt = mk("NEWTON_RS_ANT",
              Src1 * (C0 - C1 * Src0 * sq(Src1)),
              lambda in0, in1, s0, s1, imm2:
              (in1.astype(np.float32)
               * (s0 - s1 * in0 * in1 * in1)))
    spc = mk("SPCOMB_ANT",
             Src0 + C0 - C0 * sq(Src1),
             lambda in0, in1, s0, s1, imm2:
             (in0.astype(np.float32) + s0 - s0 * in1 * in1))
    return var_op, newt, spc


def _patch_act_tables():
    """Pin every activation we use (Silu/Tanh/Square/Copy/Identity) to the
    single silu_and_others table so one hoisted ACT table load serves the
    whole kernel."""
    import concourse.mybir as mybir
    import concourse.bacc as bacc_mod
    import concourse.hw_specs as hw
    if getattr(bacc_mod, "_ant_act_tables_patched", False):
        return
    orig = hw.get_activation_tables
    AF = mybir.ActivationFunctionType
    keep = "silu_and_others"
    mine = {AF.Silu, AF.Tanh, AF.Square, AF.Copy, AF.Identity}

    def patched(arch):
        t = dict(orig(arch))
        for name in list(t.keys()):
            if name != keep:
                t[name] = t[name] - mine
        return t

    bacc_mod.get_activation_tables = patched
    bacc_mod._ant_act_tables_patched = True


def _build_program():
    import concourse.mybir as mybir
    from concourse.bacc import Bacc
    from concourse.tile import TileContext
    from concourse.alu_op_type import AluOpType

    _patch_act_tables()
    var_op, newt_op, spc_op = _register_customs()

    AF = mybir.ActivationFunctionType
    OP = AluOpType
    f32 = mybir.dt.float32
    f32r = mybir.dt.float32r
    bf = mybir.dt.bfloat16
    nc = Bacc()

    xin = nc.dram_tensor("xin", (C, NPIX), f32r, kind="ExternalInput")
    yout = nc.dram_tensor("yout", (C, NPIX), f32, kind="ExternalOutput")
    dw = {}
    for name, shape in [("wbigT", (C, 576)), ("xpT", (C, 128)),
                        ("wdtT", (48, 768)), ("combT", (96, 128)),
                        ("selsA", (128, 512)), ("selsB", (128, 128)),
                        ("wspT", (C, 192)), ("woT", (C, 192))]:
        dw[name] = nc.dram_tensor(name, shape, bf, kind="ExternalInput")
    dw["onescb"] = nc.dram_tensor("onescb", (C, 1), bf, kind="ExternalInput")
    dw["ident"] = nc.dram_tensor("ident", (128, 128), f32, kind="ExternalInput")
    xpm = nc.dram_tensor("xpm", (NPIX // 128, 128, C), bf,
                         kind="ExternalInput")
    dw["pcol"] = nc.dram_tensor("pcol", (128, NPCOL), f32, kind="ExternalInput")
    nrm1d = nc.dram_tensor("nrm1d", (NTILES, 2 * NT), bf, kind="Internal")
    nrm2d = nc.dram_tensor("nrm2d", (NTILES, 2 * NT), bf, kind="Internal")

    with TileContext(nc) as tc:
        with (
            tc.tile_pool(name="wpool", bufs=1) as wp,
            tc.tile_pool(name="work", bufs=1) as wk,
            tc.tile_pool(name="ps_q", bufs=3, space="PSUM") as ps_q,     # q + dt
            tc.tile_pool(name="ps_mm2", bufs=2, space="PSUM") as ps_mm2,  # gain/s/o
            tc.tile_pool(name="ps_xd", bufs=2, space="PSUM") as ps_xd,   # xd/bcm
            tc.tile_pool(name="ps_st", bufs=1, space="PSUM") as ps_st,   # stats
        ):
            # ---- load weights once ----
            wt = {}
            for name, shape in [("wbigT_lo", (128, 576)), ("wbigT_hi", (64, 576)),
                                ("xpT_lo", (128, 128)), ("xpT_hi", (64, 128)),
                                ("wdtT", (48, 768)), ("combT", (96, 128)),
                                ("selsA", (128, 512)), ("selsB", (128, 128)),
                                ("wspT_lo", (128, 192)), ("wspT_hi", (64, 192)),
                                ("woT_lo", (128, 192)), ("woT_hi", (64, 192)),
                                ("onesb_lo", (128, 1)), ("onesb_hi", (64, 1))]:
                wt[name] = wp.tile(list(shape), bf, tag=name, name=name)
            wt["ident"] = wp.tile([128, 128], f32, tag="ident", name="ident")
            wt["pcol"] = wp.tile([128, NPCOL], f32, tag="pcol", name="pcol")
            for nm, src in [("wbigT_lo", dw["wbigT"][0:128, :]),
                            ("wbigT_hi", dw["wbigT"][128:192, :]),
                            ("xpT_lo", dw["xpT"][0:128, :]),
                            ("xpT_hi", dw["xpT"][128:192, :]),
                            ("wdtT", dw["wdtT"][:]),
                            ("combT", dw["combT"][:]),
                            ("selsA", dw["selsA"][:]),
                            ("selsB", dw["selsB"][:]),
                            ("wspT_lo", dw["wspT"][0:128, :]),
                            ("wspT_hi", dw["wspT"][128:192, :]),
                            ("woT_lo", dw["woT"][0:128, :]),
                            ("woT_hi", dw["woT"][128:192, :]),
                            ("ident", dw["ident"][:]),
                            ("onesb_lo", dw["onescb"][0:128, :]),
                            ("onesb_hi", dw["onescb"][128:192, :]),
                            ("pcol", dw["pcol"][:])]:
                nc.sync.dma_start(wt[nm][:], src)

            PC = {}
            idx = 0
            for m in range(6):
                PC[f"b{m}"] = idx; idx += 1
            for j in range(6):
                PC[f"dtb{j}"] = idx; idx += 1
                PC[f"dtbc{j}"] = idx; idx += 1
            for nm in ["DsA", "DsB", "obpA", "obpB"]:
                PC[nm] = idx; idx += 1

            def pc(name, rows=128):
                return wt["pcol"][0:rows, PC[name]:PC[name] + 1]

            def batched_rsqrt(batch, rows, seed, iters, tag, uid,
                              r0=0, r1=None, nrm=None):
                """batch [rows,1024] f32 (cols 0:512 sum(x), 512:1024
                sum(x^2)); processes row slice [r0:r1]. Returns nrm
                [rows,1024] bf (cols 0:512 rstd, 512:1024 mu*rstd)."""
                if r1 is None:
                    r1 = rows
                rs = slice(r0, r1)
                mu = wk.tile([rows, NT], f32, tag=f"mu{tag}",
                             name=f"mu{uid}", bufs=1)
                nc.vector.tensor_scalar_mul(mu[rs, :], batch[rs, 0:NT],
                                            1.0 / C)
                var = wk.tile([rows, NT], f32, tag=f"var{tag}",
                              name=f"var{uid}", bufs=1)
                nc.vector._custom_dve(var_op, out=var[rs, :],
                                      in0=batch[rs, NT:2 * NT], in1=mu[rs, :],
                                      s0=1.0 / C, s1=EPS)
                y0 = wk.tile([rows, NT], f32, tag=f"nt{tag}",
                             name=f"y0{uid}", bufs=2)
                nc.vector.tensor_scalar(y0[rs, :], var[rs, :],
                                        -0.5 * seed ** 3,
                                        1.5 * seed, OP.mult, OP.add)
                cur = y0
                if nrm is None:
                    nrm = wk.tile([rows, 2 * NT], bf, tag=f"nrm{tag}",
                                  name=f"nrm{uid}", bufs=2)
                for i in range(iters):
                    last = (i == iters - 1)
                    if last:
                        nc.vector._custom_dve(newt_op, out=nrm[rs, 0:NT],
                                              in0=var[rs, :], in1=cur[rs, :],
                                              s0=1.5, s1=0.5)
                    else:
                        yn = wk.tile([rows, NT], f32, tag=f"nt{tag}",
                                     name=f"y{i + 1}{uid}", bufs=2)
                        nc.vector._custom_dve(newt_op, out=yn[rs, :],
                                              in0=var[rs, :], in1=cur[rs, :],
                                              s0=1.5, s1=0.5)
                        cur = yn
                nc.vector.tensor_tensor(nrm[rs, NT:2 * NT], mu[rs, :],
                                        nrm[rs, 0:NT], OP.mult)
                return nrm

            sx2as, sx2bs, zas, zbs, yas, ybs = {}, {}, {}, {}, {}, {}

            # (LN1 is computed per tile inside phaseC via pixel-major
            # DVE reduces; no prologue needed.)

            # ================= MAIN LOOP =================
            def ln1_tile(t):
                # ---- LN1 stats from pixel-major x (DVE/GPS reduces) ----
                xp = wk.tile([128, 4 * C], bf, tag="xp", name=f"xp{t}", bufs=3)
                nc.sync.dma_start(
                    xp[:], xpm[4 * t:4 * t + 4, :, :].rearrange(
                        "j p c -> p j c"))
                xp3 = xp[:].rearrange("p (j c) -> p j c", j=4)
                sqp = wk.tile([128, 4 * C], bf, tag="sqp", name=f"sqp{t}",
                              bufs=2)
                nc.gpsimd.tensor_mul(sqp[:], xp[:], xp[:])
                ssum = wk.tile([128, 4], f32, tag="ssum", name=f"ssum{t}",
                               bufs=2)
                nc.vector.tensor_reduce(ssum[:].unsqueeze(2),
                                        xp3, mybir.AxisListType.X, OP.add)
                qsum = wk.tile([128, 4], f32, tag="qsum", name=f"qsum{t}",
                               bufs=2)
                nc.vector.tensor_reduce(
                    qsum[:].unsqueeze(2),
                    sqp[:].rearrange("p (j c) -> p j c", j=4),
                    mybir.AxisListType.X, OP.add)
                mu = wk.tile([128, 4], f32, tag="mus", name=f"mus{t}", bufs=2)
                nc.vector.tensor_scalar_mul(mu[:], ssum[:], 1.0 / C)
                var = wk.tile([128, 4], f32, tag="vars", name=f"vars{t}",
                              bufs=2)
                nc.vector._custom_dve(var_op, out=var[:], in0=qsum[:],
                                      in1=mu[:], s0=1.0 / C, s1=EPS)
                cur = wk.tile([128, 4], f32, tag="nts", name=f"y0s{t}", bufs=2)
                nc.vector.tensor_scalar(cur[:], var[:], -0.5 * SEED1 ** 3,
                                        1.5 * SEED1, OP.mult, OP.add)
                pk = wk.tile([128, 8], f32, tag="pks", name=f"pks{t}", bufs=2)
                for it in range(IT1):
                    dst = (pk[:, 0:4] if it == IT1 - 1 else
                           wk.tile([128, 4], f32, tag="nts",
                                   name=f"y{it + 1}s{t}", bufs=2)[:])
                    nc.vector._custom_dve(newt_op, out=dst, in0=var[:],
                                          in1=cur[:], s0=1.5, s1=0.5)
                    cur = dst
                nc.vector.tensor_tensor(pk[:, 4:8], mu[:], pk[:, 0:4],
                                        OP.mult)
                ptr = ps_st.tile([8, 128], f32, tag="st", name=f"ptr{t}")
                nc.tensor.transpose(ptr[:], pk[:], wt["ident"][:])
                ntr = wk.tile([8, 128], bf, tag="ntr", name=f"ntr{t}", bufs=2)
                nc.scalar.copy(ntr[:], ptr[:])
                nc.scalar.dma_start(
                    nrm1d[t:t + 1, 0:NT].rearrange("o (j p) -> o j p", j=4),
                    ntr[0:4, :])
                nc.scalar.dma_start(
                    nrm1d[t:t + 1, NT:2 * NT].rearrange("o (j p) -> o j p",
                                                        j=4),
                    ntr[4:8, :])


            for t in range(NTILES):
                ln1_tile(t)

            def phaseC(t, i, batch2):
                ts = slice(t * NT, (t + 1) * NT)
                xc = wk.tile([128, NT], f32r, tag="xl", name=f"xc{t}", bufs=6)
                xch = wk.tile([64, NT], f32r, tag="xh", name=f"xch{t}",
                              bufs=6)
                nc.sync.dma_start(xc[:], xin[0:128, ts])
                nc.sync.dma_start(xch[:], xin[128:192, ts])

                rb = wk.tile([128, NT], bf, tag="rb", name=f"rb{t}", bufs=2)
                mb = wk.tile([128, NT], bf, tag="mb", name=f"mb{t}", bufs=2)
                nc.gpsimd.dma_start(
                    rb[:], nrm1d[t:t + 1, 0:NT].broadcast_to([128, NT]))
                nc.gpsimd.dma_start(
                    mb[:], nrm1d[t:t + 1, NT:2 * NT].broadcast_to([128, NT]))

                # x_hat = x*rstd - mu*rstd
                xt = wk.tile([128, NT], bf, tag="xt", name=f"xt{t}", bufs=2)
                xhl = wk.tile([128, NT], bf, tag="xhl", name=f"xhl{t}", bufs=2)
                xhh = wk.tile([64, NT], bf, tag="xhh", name=f"xhh{t}", bufs=2)
                nc.vector.tensor_tensor(xt[:], xc[:].bitcast(f32), rb[:],
                                        OP.mult)
                nc.vector.tensor_tensor(xhl[:], xt[:], mb[:], OP.subtract)
                nc.vector.tensor_tensor(xt[0:64, :], xch[:].bitcast(f32),
                                        rb[0:64, :], OP.mult)
                nc.vector.tensor_tensor(xhh[:], xt[0:64, :], mb[0:64, :],
                                        OP.subtract)

                # big matmul; silu fused into psum evac
                souts = []
                for m, (r0, r1) in enumerate(QCH):
                    rows = r1 - r0
                    q = ps_q.tile([rows, NT], f32, tag="q", name=f"q{m}_{t}")
                    nc.tensor.matmul(q[:], wt["wbigT_lo"][:, r0:r1], xhl[:],
                                     start=True, stop=False)
                    nc.tensor.matmul(q[:], wt["wbigT_hi"][:, r0:r1], xhh[:],
                                     start=False, stop=True)
                    so = wk.tile([128, NT], bf, tag=f"sil{m}",
                                 name=f"sil{m}_{t}",
                                 bufs=2 if m in (0, 1) else 9)
                    nc.scalar.activation(so[0:rows, :], q[:], AF.Silu,
                                         bias=pc(f"b{m}", rows))
                    souts.append(so)
                xssA, xssB = souts[0], souts[1]
                zas[t], zbs[t] = souts[2], souts[3]
                sx2as[t], sx2bs[t] = souts[4], souts[5]

                # xd = [dtr;pad;Bv;Cv] @ xss  (one 128-row matmul)
                xdp = ps_xd.tile([128, NT], f32, tag="xd", name=f"xd{t}")
                nc.tensor.matmul(xdp[:], wt["xpT_lo"][:], xssA[0:128, :],
                                 start=True, stop=False)
                nc.tensor.matmul(xdp[:], wt["xpT_hi"][:], xssB[0:64, :],
                                 start=False, stop=True)
                xds = wk.tile([128, NT], bf, tag="xds", name=f"xds{t}", bufs=2)
                nc.scalar.copy(xds[:], xdp[:])
                # realign Cv (rows 96:128) onto Bv's partitions 64:96
                cvt = wk.tile([96, NT], bf, tag="cvt", name=f"cvt{t}", bufs=2)
                nc.sync.dma_start(cvt[64:96, :], xds[96:128, :])
                bcpt = wk.tile([96, NT], bf, tag="bcpt", name=f"bcpt{t}",
                               bufs=2)
                nc.vector.tensor_tensor(bcpt[64:96, :], xds[64:96, :],
                                        cvt[64:96, :], OP.mult)
                bcm_ps = ps_xd.tile([128, NT], f32, tag="xd",
                                    name=f"bcm_ps{t}")
                nc.tensor.matmul(bcm_ps[:], wt["combT"][64:96, :],
                                 bcpt[64:96, :])
                bcm = wk.tile([128, NT], bf, tag="bcm", name=f"bcm{t}", bufs=2)
                nc.scalar.copy(bcm[:], bcm_ps[:])

                # dt chunks -> softplus (silu + tanh correction) -> *bc
                pj = []
                for j in range(6):
                    dtp = ps_q.tile([128, NT], f32, tag="q", name=f"dt{j}_{t}")
                    nc.tensor.matmul(dtp[:],
                                     wt["wdtT"][:, 128 * j:128 * (j + 1)],
                                     xds[0:48, :])
                    sp = wk.tile([128, NT], bf, tag="sp", name=f"sp{j}_{t}",
                                 bufs=2)
                    nc.scalar.activation(sp[:], dtp[:], AF.Silu,
                                         bias=pc(f"dtb{j}"))
                    th = wk.tile([128, NT], bf, tag="th", name=f"th{j}_{t}",
                                 bufs=2)
                    nc.scalar.activation(th[:], dtp[:], AF.Tanh,
                                         bias=pc(f"dtbc{j}"), scale=SP_C)
                    spc = wk.tile([128, NT], bf, tag="spc", name=f"spc{j}_{t}",
                                  bufs=2)
                    nc.vector._custom_dve(spc_op, out=spc[:], in0=sp[:],
                                          in1=th[:], s0=SP_A)
                    pp = wk.tile([128, NT], bf, tag=f"pj{j % 3}",
                                 name=f"pj{j}_{t}", bufs=2)
                    if j % 2 == 0:
                        nc.vector.tensor_tensor(pp[:], spc[:], bcm[:], OP.mult)
                    else:
                        nc.gpsimd.tensor_mul(pp[:], spc[:], bcm[:])
                    pj.append(pp)

                gA = ps_xd.tile([128, NT], f32, tag="xd", name=f"gA{t}")
                for j in range(4):
                    nc.tensor.matmul(gA[:],
                                     wt["selsA"][:, 128 * j:128 * (j + 1)],
                                     pj[j][:], start=(j == 0), stop=(j == 3))
                gB = ps_xd.tile([64, NT], f32, tag="xd", name=f"gB{t}")
                for j in range(2):
                    nc.tensor.matmul(gB[:],
                                     wt["selsB"][:, 64 * j:64 * (j + 1)],
                                     pj[4 + j][:], start=(j == 0),
                                     stop=(j == 1))

                # y = (gain + Dsum) * xss ; LN2 stats
                yA = wk.tile([128, NT], bf, tag="yA", name=f"yA{t}", bufs=9)
                yB = wk.tile([64, NT], bf, tag="yB", name=f"yB{t}", bufs=9)
                nc.vector.scalar_tensor_tensor(yA[:], gA[:], pc("DsA"),
                                               xssA[0:128, :], OP.add, OP.mult)
                nc.vector.scalar_tensor_tensor(yB[:], gB[:], pc("DsB", 64),
                                               xssB[0:64, :], OP.add, OP.mult)
                yas[t], ybs[t] = yA, yB
                yqA = wk.tile([128, NT], bf, tag="yqA", name=f"yqA{t}", bufs=2)
                yqB = wk.tile([64, NT], bf, tag="yqB", name=f"yqB{t}", bufs=2)
                nc.scalar.activation(yqA[:], yA[:], AF.Square)
                nc.scalar.activation(yqB[:], yB[:], AF.Square)
                stp2 = ps_st.tile([33, NT], f32, tag="st", name=f"st2_{t}")
                nc.tensor.matmul(stp2[0:1, :], wt["onesb_lo"][:], yA[:],
                                 start=True, stop=False)
                nc.tensor.matmul(stp2[0:1, :], wt["onesb_hi"][:], yB[:],
                                 start=False, stop=True)
                nc.tensor.matmul(stp2[32:33, :], wt["onesb_lo"][:], yqA[:],
                                 start=True, stop=False)
                nc.tensor.matmul(stp2[32:33, :], wt["onesb_hi"][:], yqB[:],
                                 start=False, stop=True)
                ste2 = wk.tile([33, NT], f32, tag="ste", name=f"ste2_{t}",
                               bufs=2)
                nc.scalar.copy(ste2[:], stp2[:])
                nc.scalar.dma_start(batch2[i:i + 1, 0:NT], ste2[0:1, :])
                nc.scalar.dma_start(batch2[i:i + 1, NT:2 * NT],
                                    ste2[32:33, :])

            def phaseD(g, batch2):
                nrm2 = batched_rsqrt(batch2, GRP, SEED2, IT2, "2", f"2_{g}")
                nc.gpsimd.dma_start(nrm2d[g * GRP:(g + 1) * GRP, :], nrm2[:])

            def phaseE(t):
                ts = slice(t * NT, (t + 1) * NT)
                xr = wk.tile([128, NT], f32r, tag="xl", name=f"xr{t}", bufs=6)
                xrh = wk.tile([64, NT], f32r, tag="xh", name=f"xrh{t}",
                              bufs=6)
                nc.sync.dma_start(xr[:], xin[0:128, ts])
                nc.sync.dma_start(xrh[:], xin[128:192, ts])
                rb2 = wk.tile([128, NT], bf, tag="rb2", name=f"rb2_{t}",
                              bufs=2)
                mb2 = wk.tile([128, NT], bf, tag="mb2", name=f"mb2_{t}",
                              bufs=2)
                nc.gpsimd.dma_start(
                    rb2[:], nrm2d[t:t + 1, 0:NT].broadcast_to([128, NT]))
                nc.gpsimd.dma_start(
                    mb2[:], nrm2d[t:t + 1, NT:2 * NT].broadcast_to([128, NT]))

                # w = (mb2 - obp) - y*rstd2 ; v = w*z  (sign folded in Wsp)
                tA = wk.tile([128, NT], bf, tag="tA", name=f"tA{t}", bufs=2)
                tB = wk.tile([64, NT], bf, tag="tB", name=f"tB{t}", bufs=2)
                nc.vector.tensor_tensor(tA[:], yas[t][:], rb2[:], OP.mult)
                nc.gpsimd.tensor_mul(tB[:], ybs[t][:], rb2[0:64, :])
                wA = wk.tile([128, NT], bf, tag="wA", name=f"wA{t}", bufs=2)
                wB = wk.tile([64, NT], bf, tag="wB", name=f"wB{t}", bufs=2)
                nc.vector.scalar_tensor_tensor(wA[:], mb2[:], pc("obpA"),
                                               tA[:], OP.subtract, OP.subtract)
                nc.vector.scalar_tensor_tensor(wB[:], mb2[0:64, :],
                                               pc("obpB", 64), tB[:],
                                               OP.subtract, OP.subtract)
                vA = wk.tile([128, NT], bf, tag="vA", name=f"vA{t}", bufs=2)
                vB = wk.tile([64, NT], bf, tag="vB", name=f"vB{t}", bufs=2)
                nc.vector.tensor_tensor(vA[:], wA[:], zas[t][0:128, :],
                                        OP.mult)
                nc.vector.tensor_tensor(vB[:], wB[:], zbs[t][0:64, :], OP.mult)

                # s = (-Wsp') @ v
                sA = ps_mm2.tile([128, NT], f32, tag="mm2", name=f"sA{t}")
                sB = ps_mm2.tile([64, NT], f32, tag="mm2", name=f"sB{t}")
                nc.tensor.matmul(sA[:], wt["wspT_lo"][:, 0:128], vA[:],
                                 start=True, stop=False)
                nc.tensor.matmul(sA[:], wt["wspT_hi"][:, 0:128], vB[:],
                                 start=False, stop=True)
                nc.tensor.matmul(sB[:], wt["wspT_lo"][:, 128:192], vA[:],
                                 start=True, stop=False)
                nc.tensor.matmul(sB[:], wt["wspT_hi"][:, 128:192], vB[:],
                                 start=False, stop=True)

                gsA = wk.tile([128, NT], bf, tag="gsA", name=f"gsA{t}", bufs=2)
                gsB = wk.tile([64, NT], bf, tag="gsB", name=f"gsB{t}", bufs=2)
                nc.vector.tensor_tensor(gsA[:], sA[:], sx2as[t][0:128, :],
                                        OP.mult)
                nc.vector.tensor_tensor(gsB[:], sB[:], sx2bs[t][0:64, :],
                                        OP.mult)

                oA = ps_mm2.tile([128, NT], f32, tag="mm2", name=f"oA{t}")
                oB = ps_mm2.tile([64, NT], f32, tag="mm2", name=f"oB{t}")
                nc.tensor.matmul(oA[:], wt["woT_lo"][:, 0:128], gsA[:],
                                 start=True, stop=False)
                nc.tensor.matmul(oA[:], wt["woT_hi"][:, 0:128], gsB[:],
                                 start=False, stop=True)
                nc.tensor.matmul(oB[:], wt["woT_lo"][:, 128:192], gsA[:],
                                 start=True, stop=False)
                nc.tensor.matmul(oB[:], wt["woT_hi"][:, 128:192], gsB[:],
                                 start=False, stop=True)

                outA = wk.tile([128, NT], f32, tag="outA", name=f"outA{t}",
                               bufs=2)
                outB = wk.tile([64, NT], f32, tag="outB", name=f"outB{t}",
                               bufs=2)
                nc.vector.tensor_tensor(outA[:], oA[:], xr[:].bitcast(f32),
                                        OP.add)
                nc.vector.tensor_tensor(outB[:], oB[:], xrh[:].bitcast(f32),
                                        OP.add)
                nc.sync.dma_start(yout[0:128, ts], outA[:])
                nc.sync.dma_start(yout[128:192, ts], outB[:])

            batches = {}
            for g in range(NGRP):
                batch2 = wk.tile([GRP, 2 * NT], f32, tag="batch2",
                                 name=f"batch2_{g}", bufs=2)
                batches[g] = batch2
                for i in range(GRP):
                    phaseC(g * GRP + i, i, batch2)
                if g > 0:
                    for i in range(GRP):
                        phaseE((g - 1) * GRP + i)
                phaseD(g, batch2)
            for i in range(GRP):
                phaseE((NGRP - 1) * GRP + i)

    nc.compile()
    return nc


def _get_program():
    if "nc" not in _CACHE:
        _CACHE["nc"] = _build_program()
    return _CACHE["nc"]


def _shard(x):
    """x (B,C,H,W) -> list of 8 (C, NPIX) channel-major slices"""
    outs = []
    for i in range(NCORES):
        b, h0 = i // 2, (i % 2) * (H // 2)
        outs.append(np.ascontiguousarray(
            x[b, :, h0:h0 + H // 2, :].reshape(C, NPIX)))
    return outs


def _shard_pm(shards):
    """channel-major shards -> pixel-major bf16 (NPIX//128, 128, C)"""
    import ml_dtypes
    outs = []
    for s in shards:
        pm = np.ascontiguousarray(s.T).astype(ml_dtypes.bfloat16)
        outs.append(pm.reshape(NPIX // 128, 128, C))
    return outs


def _unshard(parts, dtype):
    out = np.empty((B, C, H, W), dtype)
    for i in range(NCORES):
        b, h0 = i // 2, (i % 2) * (H // 2)
        out[b, :, h0:h0 + H // 2, :] = parts[i].reshape(C, H // 2, W)
    return out


def run(inputs, trace=False):
    from concourse.bass_utils import run_bass_kernel_spmd
    nc = _get_program()
    wts, _ = _fold_weights(**{k: np.asarray(v) for k, v in inputs.items()
                              if k != "x"})
    shards = _shard(np.asarray(inputs["x"], np.float32))
    pms = _shard_pm(shards)
    in_maps = [dict(wts, xin=s, xpm=p) for s, p in zip(shards, pms)]
    res = run_bass_kernel_spmd(nc, in_maps, core_ids=list(range(NCORES)),
                               trace=trace)
    out = _unshard([r["yout"] for r in res.results], np.float32)
    return out, res


def kernel(**inputs):
    out, _ = run(inputs, trace=False)
    return out


if __name__ == "__main__":
    print("building program...")
    nc = _get_program()
    print("program built")
